# revision 33
# baseline (speedup 1.0000x reference)
"""GCN graph classifier on 8 Trainium2 NeuronCores (Bass/Tile).

Strategy (graph/data parallel per the sharding hint):
- Nodes are split into 8 contiguous ranges aligned to graph boundaries; each
  core owns the destination side of every edge landing in its range (plus one
  self-loop edge per owned node), pooling and the MLP head for its graphs.
- Message passing is computed as agg = dinv * ((S+I) @ (dinv * h)) via
  per-edge row gathers (dma_gather, 256B bf16 rows) and one-hot matmuls that
  scatter-accumulate each 128-edge chunk into its 128-node window in PSUM.
- Two launches: layer 1 (aggregates 2-wide x, emits bf16 dinv*relu(conv1)
  rows), host reassembles the full hidden table, layer 2 + mean-pool + head.
- Tables are bf16 with 256B rows (the dma_gather element granularity), so
  gathered rows feed the scatter matmuls directly with no per-edge cast.
- One-hot builds (the dominant element-wise work) are split between the
  Vector and GpSimd engines; PSUM evacuations ride the Activation engine;
  conv bias is folded into the weight matmul via an appended ones row.
- All per-core variation (indices, one-hot selectors, degree data) is input
  data; the compiled program is identical across cores (SPMD).

Self-contained: no imports from the problem directory.
"""
import functools
import time

import numpy as np

import concourse.bass as bass
import concourse.bacc as bacc
import concourse.mybir as mybir
import concourse.tile as tile

N_NODES = 100_000
N_PAD = 100_352            # 128-aligned, 3*32768 + 2048
N_EDGES = 1_200_000
N_GRAPHS = 512
HID = 64
TBLW = 128                 # table row width in bf16 (256B = gather elem)
NCORES = 8
P = 128
CHUNK_ROWS = 32_768        # int16-addressable table chunk (window width)
NCHUNK = 4                 # gather windows into the (rotated) table
# Edge->chunk assignment boundaries and window start rows.  Each core's table
# copy is ROTATED so its own nodes sit at rows [0, n1-n0): self-loop rows are
# then at compile-time offsets (contiguous DMA, no gather), and every core
# sees the same uniform source distribution, which lets the chunk boundaries
# be tuned for minimal 128-padding (K per (slot,chunk) piece ~ 4+4+3+3).
CH_BOUNDS = [28_928, 57_856, 79_104]
CH_STARTS = [0, 28_928, 57_856, N_PAD - CHUNK_ROWS]
BATCH_SLOTS = 4            # slots per gather batch
GMAX = 96                  # padded graphs per core (real ~64)
SENT = 30_000.0            # one-hot sentinel (never matches iota 0..127)
_PAD_SPREAD = True         # pad gather slots: spread over 2048 rows vs row 0
OH_DVE_FRAC = 1.0          # fraction of one-hot build columns on DVE (rest Pool;
                           # plain TensorTensor is rejected on Pool by walrus)
ROTATE_QUEUES = True       # balance SWDGE queues across unequal chunk calls
SINGLE_PACKET = False      # dma_gather single_packet flag

F32 = mybir.dt.float32
BF16 = mybir.dt.bfloat16
I16 = mybir.dt.int16


# ---------------------------------------------------------------- host prep

def _prep(edge_index: np.ndarray, batch: np.ndarray):
    """All index-side preprocessing (sharding metadata). No float math on
    values here - only integer index manipulation derived from the graph
    structure, plus integer degree counts (rsqrt happens on device)."""
    src = np.asarray(edge_index[0], dtype=np.int64)
    dst = np.asarray(edge_index[1], dtype=np.int64)
    batch = np.asarray(batch, dtype=np.int64)

    deg = np.bincount(dst, minlength=N_NODES) + 1  # int degree (self-loop +1)

    gptr = np.searchsorted(batch, np.arange(N_GRAPHS + 1))
    targets = (np.arange(1, NCORES) * N_NODES) // NCORES
    gsplit = np.searchsorted(gptr, targets)
    g0 = np.concatenate([[0], gsplit, [N_GRAPHS]])
    n0s = gptr[g0]  # node range starts per core (len 9)

    order = np.argsort(dst, kind="stable")
    dst_s = dst[order]
    src_s = src[order]
    e0s = np.searchsorted(dst_s, n0s)

    cores = []
    for c in range(NCORES):
        n0, n1 = int(n0s[c]), int(n0s[c + 1])
        eso = src_s[e0s[c]:e0s[c + 1]]      # original (global) source ids
        ed = dst_s[e0s[c]:e0s[c + 1]]
        # self-loop edges are NOT in the gather stream: they are served by a
        # contiguous read of the core's own (rotation-fronted) table rows
        es = (eso - n0) % N_PAD             # rotated source rows
        slot = (ed - n0) >> 7
        chunk = np.searchsorted(np.array(CH_BOUNDS), es, side="right")
        o2 = np.lexsort((slot, chunk, slot >> 3))  # (batch, chunk, slot)
        cores.append({
            "n0": n0, "n1": n1, "g0": int(g0[c]), "g1": int(g0[c + 1]),
            "es": es[o2], "eso": eso[o2], "ed": ed[o2],
            "slot": slot[o2], "chunk": chunk[o2],
            "W": int(-(-(n1 - n0) // P)),
        })

    W_SLOTS = max(cr["W"] for cr in cores)
    W_SLOTS = -(-W_SLOTS // BATCH_SLOTS) * BATCH_SLOTS  # pad to batch multiple
    NB = W_SLOTS // BATCH_SLOTS

    # per (slot, chunk) 128-block counts, cross-core max
    counts = np.zeros((NCORES, W_SLOTS, NCHUNK), dtype=np.int64)
    for c, cr in enumerate(cores):
        key = cr["slot"] * NCHUNK + cr["chunk"]
        bc = np.bincount(key, minlength=W_SLOTS * NCHUNK)
        counts[c] = bc.reshape(W_SLOTS, NCHUNK)
    K = np.maximum(-(-counts.max(axis=0) // P), 1)  # [W_SLOTS, NCHUNK] blocks

    # block layout: ordered by (batch, chunk, slot-in-batch, k)
    # block_base[s, ch] = index of first block of that piece
    block_base = np.zeros((W_SLOTS, NCHUNK), dtype=np.int64)
    call_meta = []  # per gather call: (chunk, edge_base, n_edges)
    nb_blocks = 0
    for b in range(NB):
        sl = slice(b * BATCH_SLOTS, (b + 1) * BATCH_SLOTS)
        for ch in range(NCHUNK):
            kb = K[sl, ch]
            block_base[sl, ch] = nb_blocks + np.concatenate([[0], np.cumsum(kb)[:-1]])
            ncall = int(kb.sum()) * P
            call_meta.append((ch, nb_blocks * P, ncall))
            nb_blocks += int(kb.sum())
    NSUB = nb_blocks
    NE_LAY = NSUB * P  # total gather slots per layer

    # per-sub (block) metadata: slot position + start/stop flags, slot-major
    sub_of = []  # in processing order: (sub_idx, slot, is_first, is_last)
    for b in range(NB):
        for s in range(b * BATCH_SLOTS, (b + 1) * BATCH_SLOTS):
            subs = []
            for ch in range(NCHUNK):
                for k in range(int(K[s, ch])):
                    subs.append(int(block_base[s, ch]) + k)
            for i, sub in enumerate(subs):
                sub_of.append((sub, s, i == 0, i == len(subs) - 1))

    # per-core data arrays
    percore = []
    for c, cr in enumerate(cores):
        es, ed, slot, chunk = cr["es"], cr["ed"], cr["slot"], cr["chunk"]
        eso = cr["eso"]
        key = slot * NCHUNK + chunk
        grp_order = np.lexsort((np.arange(len(es)), key))  # already sorted but safe
        # rank within (slot, chunk) group
        sort_key = key[grp_order]
        ranks = np.arange(len(es)) - np.searchsorted(sort_key, sort_key)
        # position of each edge
        pos = block_base[slot[grp_order], chunk[grp_order]] * P + ranks
        esg = es[grp_order]
        esog = eso[grp_order]
        edg = ed[grp_order]

        # pad slots gather garbage rows (masked by the one-hot sentinel);
        # spread them across each chunk's rows to avoid hammering one HBM
        # row with thousands of identical descriptors
        rng_pad = np.random.default_rng(12345)
        if _PAD_SPREAD:
            # spread pad descriptors across each call's whole chunk (the last
            # chunk is short); identical/clustered pad rows serialize the DMA
            # engines on HBM row conflicts
            idx_flat = np.empty(NE_LAY, dtype=np.int16)
            for ch_, ebase_, ncall_ in call_meta:
                idx_flat[ebase_:ebase_ + ncall_] = rng_pad.integers(
                    0, CHUNK_ROWS, size=ncall_).astype(np.int16)
        else:
            idx_flat = np.zeros(NE_LAY, dtype=np.int16)
        idx_flat[pos] = (esg - np.array(CH_STARTS)[chunk[grp_order]]).astype(np.int16)

        dst_rel = np.full((P, NSUB), SENT, dtype=np.float32)
        dst_rel[pos % P, pos >> 7] = (edg - cr["n0"] - slot[grp_order] * P).astype(np.float32)
        # slot-major column order so each slot's blocks are contiguous (the
        # batched one-hot build slices a contiguous range per slot)
        dst_rel = dst_rel[:, [so[0] for so in sub_of]]
        dst_rel = dst_rel.astype(mybir.dt.np(BF16))  # exact for 0..127 + sentinel

        deg_prod = np.ones((P, NSUB), dtype=np.float32)
        deg_prod[pos % P, pos >> 7] = (deg[esog] * deg[edg]).astype(np.float32)

        nown = cr["n1"] - cr["n0"]
        ar = np.arange(nown)
        deg_own = np.ones((P, W_SLOTS), dtype=np.float32)
        deg_own[ar % P, ar >> 7] = deg[cr["n0"]:cr["n1"]].astype(np.float32)

        g_rel = np.full((P, W_SLOTS), SENT, dtype=np.float32)
        g_rel[ar % P, ar >> 7] = (batch[cr["n0"]:cr["n1"]] - cr["g0"]).astype(np.float32)
        g_rel = g_rel.astype(mybir.dt.np(BF16))  # exact (values < 256 + sentinel)

        # pack idx into [128, NE_LAY//16] int16 col-major-16 replicated
        cols = NE_LAY // 16
        arr = np.zeros((16, cols), dtype=np.int16)
        j = np.arange(NE_LAY)
        arr[j % 16, j // 16] = idx_flat
        idx_packed = np.tile(arr, (8, 1))

        percore.append({
            **{k: cr[k] for k in ("n0", "n1", "g0", "g1", "W")},
            "idx_packed": idx_packed, "dst_rel": dst_rel,
            "deg_prod": deg_prod, "deg_own": deg_own, "g_rel": g_rel,
        })

    struct = {
        "W_SLOTS": W_SLOTS, "NB": NB, "NSUB": NSUB, "NE_LAY": NE_LAY,
        "K": K, "block_base": block_base, "call_meta": call_meta,
        "sub_of": sub_of,
    }
    return struct, percore, deg, gptr, n0s, g0


# ------------------------------------------------------------- bass program

def _build_launch(struct, layer: int, reps: int = 1, timing: bool = False, part: str = 'all'):
    """Build the SPMD Bass program for layer 1 or layer 2(+pool+mlp)."""
    W_SLOTS, NB = struct["W_SLOTS"], struct["NB"]
    NSUB, NE_LAY = struct["NSUB"], struct["NE_LAY"]
    K, block_base = struct["K"], struct["block_base"]
    call_meta, sub_of = struct["call_meta"], struct["sub_of"]

    nc = bacc.Bacc("TRN2", num_swdge_queues=4)
    table = nc.dram_tensor("table", (N_PAD, TBLW), BF16, kind="ExternalInput")
    idx_in = nc.dram_tensor("idx", (P, NE_LAY // 16), I16, kind="ExternalInput")
    dst_rel_in = nc.dram_tensor("dst_rel", (P, NSUB), BF16, kind="ExternalInput")
    deg_own_in = nc.dram_tensor("deg_own", (P, W_SLOTS), F32, kind="ExternalInput")
    iota_in = nc.dram_tensor("iota", (P, P), F32, kind="ExternalInput")
    ones_in = nc.dram_tensor("ones1", (1, P), F32, kind="ExternalInput")
    ident_in = nc.dram_tensor("ident", (P, P), F32, kind="ExternalInput")
    msg_w = 2 if layer == 1 else HID
    # conv weight with the bias folded in as a final row ([W; b])
    w_in = nc.dram_tensor("Wc", (msg_w + 1, HID), F32, kind="ExternalInput")
    if layer == 1:
        deg_prod_in = nc.dram_tensor("deg_prod", (P, NSUB), F32, kind="ExternalInput")
        if timing:
            out_t = nc.dram_tensor("h1s_scratch", (W_SLOTS * P, HID), BF16)
            dummy_t = nc.dram_tensor("tdummy0", (1, 4), F32, kind="ExternalOutput")
        else:
            out_t = nc.dram_tensor("h1s", (W_SLOTS * P, HID), BF16, kind="ExternalOutput")
    else:
        g_rel_in = nc.dram_tensor("g_rel", (P, W_SLOTS), BF16, kind="ExternalInput")
        deg_row_in = nc.dram_tensor("deg_row", (1, W_SLOTS * P), F32, kind="ExternalInput")
        wf1_in = nc.dram_tensor("Wf1", (HID, HID), F32, kind="ExternalInput")
        bf1_in = nc.dram_tensor("bf1", (1, HID), F32, kind="ExternalInput")
        wf2_in = nc.dram_tensor("Wf2", (HID, 4), F32, kind="ExternalInput")
        bf2_in = nc.dram_tensor("bf2", (1, 4), F32, kind="ExternalInput")
        if timing:
            out_t = nc.dram_tensor("out_scratch", (GMAX, 4), F32)
            dummy_t = nc.dram_tensor("tdummy0", (1, 4), F32, kind="ExternalOutput")
        else:
            out_t = nc.dram_tensor("out", (GMAX, 4), F32, kind="ExternalOutput")

    # organize subs per slot for slot-major processing
    slot_subs = [[] for _ in range(W_SLOTS)]
    for sub, s, first, last in sub_of:
        slot_subs[s].append(sub)
    # slot-major column starts into the (permuted) dst_rel array
    slot_col0 = np.zeros(W_SLOTS, dtype=np.int64)
    acc_cols = 0
    for s in range(W_SLOTS):
        slot_col0[s] = acc_cols
        acc_cols += len(slot_subs[s])

    # sub -> (call index, block-within-call) for gather tile slicing
    sub_call = np.zeros(NSUB, dtype=np.int64)
    sub_kloc = np.zeros(NSUB, dtype=np.int64)
    for ci, (ch, ebase, ncall) in enumerate(call_meta):
        b0 = ebase // P
        nb = ncall // P
        sub_call[b0:b0 + nb] = ci
        sub_kloc[b0:b0 + nb] = np.arange(nb)

    with tile.TileContext(nc) as tc:
        with tc.tile_pool(name="const", bufs=1) as cpool, \
             tc.tile_pool(name="meta", bufs=1) as mpool, \
             tc.tile_pool(name="gat", bufs=6) as gpool, \
             tc.tile_pool(name="own", bufs=8) as opool, \
             tc.tile_pool(name="gbf", bufs=8) as gbpool, \
             tc.tile_pool(name="work", bufs=3) as wpool, \
             tc.tile_pool(name="oh", bufs=3) as ohpool, \
             tc.tile_pool(name="pacc", bufs=3, space="PSUM") as pacc, \
             tc.tile_pool(name="ptp", bufs=2, space="PSUM") as ptp, \
             tc.tile_pool(name="ppool", bufs=1, space="PSUM") as ppool:

            # ---- load constants / metadata
            iota_t = cpool.tile([P, P], F32)
            nc.sync.dma_start(out=iota_t[:], in_=iota_in[:])
            iota_bf = cpool.tile([P, P], BF16)
            nc.vector.tensor_copy(out=iota_bf[:], in_=iota_t[:])
            ident_t = cpool.tile([P, P], F32)
            nc.sync.dma_start(out=ident_t[:], in_=ident_in[:])
            ident_bf = cpool.tile([P, P], BF16)
            nc.vector.tensor_copy(out=ident_bf[:], in_=ident_t[:])
            ones_t = cpool.tile([1, P], F32)
            nc.sync.dma_start(out=ones_t[:], in_=ones_in[:])
            w_t = cpool.tile([msg_w + 1, HID], F32)
            nc.sync.dma_start(out=w_t[:], in_=w_in[:])
            b_row_t = cpool.tile([1, HID], F32)
            nc.sync.dma_start(out=b_row_t[:], in_=w_in[msg_w:msg_w + 1, :])

            idx_t = mpool.tile([P, NE_LAY // 16], I16)
            nc.sync.dma_start(out=idx_t[:], in_=idx_in[:])
            dst_rel_t = mpool.tile([P, NSUB], BF16)
            nc.sync.dma_start(out=dst_rel_t[:], in_=dst_rel_in[:])
            deg_own_t = mpool.tile([P, W_SLOTS], F32)
            nc.sync.dma_start(out=deg_own_t[:], in_=deg_own_in[:])

            # dinv2 = 1/deg (self-loop weight), dinv = rsqrt(deg)
            dinv2_own_t = mpool.tile([P, W_SLOTS], F32)
            nc.vector.reciprocal(out=dinv2_own_t[:], in_=deg_own_t[:])
            dinv_own_t = deg_own_t
            nc.scalar.sqrt(out=dinv_own_t[:], in_=dinv2_own_t[:])

            if layer == 1:
                # per-edge norm = rsqrt(deg[src] * deg[dst]), in place
                deg_prod_t = mpool.tile([P, NSUB], F32)
                nc.sync.dma_start(out=deg_prod_t[:], in_=deg_prod_in[:])
                norm_t = deg_prod_t
                nc.vector.reciprocal(out=norm_t[:], in_=deg_prod_t[:])
                nc.scalar.sqrt(out=norm_t[:], in_=norm_t[:])
                # per-slot diag(dinv^2) for the self-loop matmul (bf16)
                diag2_all = cpool.tile([P, W_SLOTS, P], BF16)
                nc.vector.tensor_tensor(
                    out=diag2_all[:],
                    in0=ident_bf[:].rearrange("p (k c) -> p k c", k=1)
                        .broadcast_to((P, W_SLOTS, P)),
                    in1=dinv2_own_t[:].rearrange("p (k o) -> p k o", o=1)
                        .broadcast_to((P, W_SLOTS, P)),
                    op=mybir.AluOpType.mult)

            if layer == 2:
                # per-slot sqrt(deg) rows for the bias matmul (bias must not
                # be scaled by the deferred dinv: h = relu(dinv*(agg@W) + b)
                # is realized as relu(dinv*(agg@W + sqrtdeg*b)))
                deg_row_t = mpool.tile([1, W_SLOTS * P], F32)
                nc.sync.dma_start(out=deg_row_t[:], in_=deg_row_in[:])
                sqdeg_row_t = deg_row_t
                nc.scalar.sqrt(out=sqdeg_row_t[:], in_=deg_row_t[:])

            if layer == 2:
                g_rel_t = mpool.tile([P, W_SLOTS], BF16)
                nc.sync.dma_start(out=g_rel_t[:], in_=g_rel_in[:])
                wf1_t = cpool.tile([HID, HID], F32)
                nc.sync.dma_start(out=wf1_t[:], in_=wf1_in[:])
                wf2_t = cpool.tile([HID, 4], F32)
                nc.sync.dma_start(out=wf2_t[:], in_=wf2_in[:])
                bf1_t = cpool.tile([1, HID], F32)
                nc.sync.dma_start(out=bf1_t[:], in_=bf1_in[:])
                bf2_t = cpool.tile([1, 4], F32)
                nc.sync.dma_start(out=bf2_t[:], in_=bf2_in[:])
                # head bias broadcasts via ones-matmul
                bb2_ps = ptp.tile([P, HID], F32, space="PSUM", tag="hps")
                nc.tensor.matmul(out=bb2_ps[:], lhsT=ones_t[:], rhs=bf1_t[:],
                                 start=True, stop=True)
                bf1_bcast = cpool.tile([P, HID], F32)
                nc.vector.tensor_copy(out=bf1_bcast[:], in_=bb2_ps[:])
                bb3_ps = ptp.tile([P, 4], F32, space="PSUM", tag="hps")
                nc.tensor.matmul(out=bb3_ps[:], lhsT=ones_t[:], rhs=bf2_t[:],
                                 start=True, stop=True)
                bf2_bcast = cpool.tile([P, 4], F32)
                nc.vector.tensor_copy(out=bf2_bcast[:], in_=bb3_ps[:])
                pool_ps = ppool.tile([GMAX, HID + 1], F32, space="PSUM")

            # ---- main loop over batches (reps>1 repeats the whole
            # edge pass for timing-slope measurement; outputs stay valid
            # because each rep re-derives the same values).  part='gather'
            # instead repeats only the dma_gather calls (WAW-chained on the
            # same tile); part='compute' repeats only the scatter/compute.
            ncall_per_b = NCHUNK
            grep = reps if part == "gather" else 1
            crep = reps if part == "compute" else 1
            arep = reps if part == "all" else 1
            for rep, b in [(r, b) for r in range(arep) for b in range(NB)]:
                gtiles = []
                for ci in range(b * ncall_per_b, (b + 1) * ncall_per_b):
                    ch, ebase, ncall = call_meta[ci]
                    g_t = gpool.tile([P, ncall // P, TBLW], BF16, tag=f"g{ci % ncall_per_b}")
                    qn = (ci + ci // 4) % 4 if ROTATE_QUEUES else ci % 4
                    for _ in range(grep):
                        nc.gpsimd.dma_gather(
                            out_ap=g_t[:],
                            in_ap=table[CH_STARTS[ch]: CH_STARTS[ch] + CHUNK_ROWS, :],
                            idxs_ap=idx_t[:, ebase // 16:(ebase + ncall) // 16],
                            num_idxs=ncall, num_idxs_reg=ncall, elem_size=TBLW,
                            single_packet=SINGLE_PACKET, queue_num=qn)
                    if layer == 1:
                        # fuse per-edge norm into the (tiny 2-wide) messages
                        b0 = ebase // P
                        g_bf = gbpool.tile([P, ncall // P, msg_w], BF16, tag="gb")
                        nc.vector.tensor_tensor(
                            out=g_bf[:], in0=g_t[:, :, 0:msg_w],
                            in1=norm_t[:, b0:b0 + ncall // P]
                                .rearrange("p (k o) -> p k o", o=1)
                                .broadcast_to((P, ncall // P, msg_w)),
                            op=mybir.AluOpType.mult)
                        gtiles.append(g_bf)
                    else:
                        gtiles.append(g_t)

                if part == "gather":
                    continue
                if layer == 2:
                    goh_big = ohpool.tile([P, BATCH_SLOTS, GMAX], BF16, tag="goh")
                    nc.vector.tensor_tensor(
                        out=goh_big[:],
                        in0=iota_bf[:, 0:GMAX]
                            .rearrange("p (k c) -> p k c", k=1)
                            .broadcast_to((P, BATCH_SLOTS, GMAX)),
                        in1=g_rel_t[:, b * BATCH_SLOTS:(b + 1) * BATCH_SLOTS]
                            .rearrange("p (k o) -> p k o", o=1)
                            .broadcast_to((P, BATCH_SLOTS, GMAX)),
                        op=mybir.AluOpType.is_equal)
                for crep_i, s in [(r, s) for r in range(crep)
                                  for s in range(b * BATCH_SLOTS, (b + 1) * BATCH_SLOTS)]:
                    subs = slot_subs[s]
                    k = len(subs)
                    col0 = slot_col0[s]
                    # accumulate the TRANSPOSED aggregate: accT[f, d] so the
                    # conv matmul consumes it directly (no PE transpose)
                    acc = pacc.tile([msg_w, P], F32, space="PSUM", tag="acc")
                    # batched 0/1 one-hots for all this slot's blocks at once
                    oh_big = ohpool.tile([P, k, P], BF16, tag="oh")
                    nc.vector.tensor_tensor(
                        out=oh_big[:],
                        in0=iota_bf[:]
                            .rearrange("p (k c) -> p k c", k=1)
                            .broadcast_to((P, k, P)),
                        in1=dst_rel_t[:, col0:col0 + k]
                            .rearrange("p (k o) -> p k o", o=1)
                            .broadcast_to((P, k, P)),
                        op=mybir.AluOpType.is_equal)
                    for i, sub in enumerate(subs):
                        ci = int(sub_call[sub])
                        kloc = int(sub_kloc[sub])
                        g_t = gtiles[ci % ncall_per_b]
                        lhs = g_t[:, kloc, :] if layer == 1 else g_t[:, kloc, 0:HID]
                        nc.tensor.matmul(
                            out=acc[:], lhsT=lhs,
                            rhs=oh_big[:, i, :],
                            start=(i == 0), stop=False)
                    # self-loop term: this core's own rows are rotation-
                    # fronted at compile-time offsets -> contiguous DMA plus
                    # an identity (L2) / diag(dinv^2) (L1) matmul
                    own_t = opool.tile([P, TBLW], BF16, tag="own")
                    nc.sync.dma_start(out=own_t[:], in_=table[s * P:(s + 1) * P, :])
                    rhs_self = diag2_all[:, s, :] if layer == 1 else ident_bf[:]
                    nc.tensor.matmul(out=acc[:], lhsT=own_t[:, 0:msg_w],
                                     rhs=rhs_self, start=False, stop=True)

                    # ---- slot tail: evacuate accT, conv matmul + bias row
                    a2t = wpool.tile([msg_w, P], F32, tag="a2t")
                    nc.scalar.copy(out=a2t[:], in_=acc[:])
                    h_ps = ptp.tile([P, HID], F32, space="PSUM", tag="hps")
                    nc.tensor.matmul(out=h_ps[:], lhsT=a2t[:], rhs=w_t[0:msg_w, :],
                                     start=True, stop=False)
                    if layer == 1:
                        nc.tensor.matmul(out=h_ps[:], lhsT=ones_t[:],
                                         rhs=b_row_t[:],
                                         start=False, stop=True)
                        h1s = wpool.tile([P, HID], BF16, tag="h1s")
                        # emit dinv[node] * relu(conv1): dinv_src prefold for
                        # the layer-2 gather (bf16 table rows)
                        nc.scalar.activation(
                            out=h1s[:], in_=h_ps[:],
                            func=mybir.ActivationFunctionType.Relu,
                            scale=dinv_own_t[:, s:s + 1])
                        nc.sync.dma_start(out=out_t[s * P:(s + 1) * P, :], in_=h1s[:])
                    else:
                        # bias row scaled by sqrt(deg) so the dinv activation
                        # scale yields relu(dinv*(agg@W) + b)
                        nc.tensor.matmul(out=h_ps[:],
                                         lhsT=sqdeg_row_t[:, s * P:(s + 1) * P],
                                         rhs=b_row_t[:],
                                         start=False, stop=True)
                        h2 = wpool.tile([P, HID + 1], BF16, tag="h2")
                        nc.scalar.activation(
                            out=h2[:, 0:HID], in_=h_ps[:],
                            func=mybir.ActivationFunctionType.Relu,
                            scale=dinv_own_t[:, s:s + 1])
                        nc.vector.memset(h2[:, HID:HID + 1], 1.0)
                        nc.tensor.matmul(out=pool_ps[:],
                                         lhsT=goh_big[:, s - b * BATCH_SLOTS, :],
                                         rhs=h2[:],
                                         start=(s == 0), stop=(s == W_SLOTS - 1))

            if timing:
                d = wpool.tile([1, 4], F32, tag="dmy")
                nc.vector.memset(d[:], 0.0)
                nc.sync.dma_start(out=dummy_t[:], in_=d[:])

            # ---- pool + MLP head (layer 2)
            if layer == 2 and part not in ("gather",):
                pool_sb = wpool.tile([GMAX, HID + 1], F32, tag="pool")
                nc.vector.tensor_copy(out=pool_sb[:], in_=pool_ps[:])
                cnt = wpool.tile([GMAX, 1], F32, tag="cnt")
                nc.vector.tensor_scalar(
                    out=cnt[:], in0=pool_sb[:, HID:HID + 1], scalar1=1.0,
                    scalar2=None, op0=mybir.AluOpType.max)
                rcnt = wpool.tile([GMAX, 1], F32, tag="rcnt")
                nc.vector.reciprocal(out=rcnt[:], in_=cnt[:])
                means = wpool.tile([GMAX, HID], F32, tag="means")
                nc.scalar.mul(means[:], pool_sb[:, 0:HID], rcnt[:])
                mt_ps = ptp.tile([HID, GMAX], F32, space="PSUM", tag="tp")
                nc.tensor.transpose(out=mt_ps[:], in_=means[:],
                                    identity=ident_t[0:GMAX, 0:GMAX])
                mt = wpool.tile([HID, GMAX], F32, tag="mt")
                nc.vector.tensor_copy(out=mt[:], in_=mt_ps[:])
                f1_ps = ptp.tile([GMAX, HID], F32, space="PSUM", tag="hps")
                nc.tensor.matmul(out=f1_ps[:], lhsT=mt[:], rhs=wf1_t[:],
                                 start=True, stop=True)
                f1 = wpool.tile([GMAX, HID], F32, tag="f1")
                nc.vector.tensor_tensor(out=f1[:], in0=f1_ps[:],
                                        in1=bf1_bcast[0:GMAX, :],
                                        op=mybir.AluOpType.add)
                nc.scalar.activation(out=f1[:], in_=f1[:],
                                     func=mybir.ActivationFunctionType.Relu)
                f1t_ps = ptp.tile([HID, GMAX], F32, space="PSUM", tag="tp")
                nc.tensor.transpose(out=f1t_ps[:], in_=f1[:],
                                    identity=ident_t[0:GMAX, 0:GMAX])
                f1t = wpool.tile([HID, GMAX], F32, tag="f1t")
                nc.vector.tensor_copy(out=f1t[:], in_=f1t_ps[:])
                o_ps = ptp.tile([GMAX, 4], F32, space="PSUM", tag="hps")
                nc.tensor.matmul(out=o_ps[:], lhsT=f1t[:], rhs=wf2_t[:],
                                 start=True, stop=True)
                o_sb = wpool.tile([GMAX, 4], F32, tag="osb")
                nc.vector.tensor_tensor(out=o_sb[:], in0=o_ps[:],
                                        in1=bf2_bcast[0:GMAX, :],
                                        op=mybir.AluOpType.add)
                nc.sync.dma_start(out=out_t[:], in_=o_sb[:])

    nc.finalize()
    return nc


# ---------------------------------------------------------------- pjrt run

class _Runner:
    def __init__(self, nc, n_cores: int = NCORES):
        import jax
        from jax.sharding import Mesh, NamedSharding, PartitionSpec
        from jax.experimental.shard_map import shard_map
        from concourse.bass2jax import (
            _bass_exec_p, install_neuronx_cc_hook, partition_id_tensor)

        install_neuronx_cc_hook()
        self.jax = jax
        self.n_cores = n_cores
        in_names, out_names, out_avals = [], [], []
        pname = nc.partition_id_tensor.name if nc.partition_id_tensor else None
        for alloc in nc.m.functions[0].allocations:
            if not isinstance(alloc, mybir.MemoryLocationSet):
                continue
            name = alloc.memorylocations[0].name
            if alloc.kind == "ExternalInput":
                if name != pname:
                    in_names.append(name)
            elif alloc.kind == "ExternalOutput":
                out_names.append(name)
                out_avals.append(jax.core.ShapedArray(
                    tuple(alloc.tensor_shape), mybir.dt.np(alloc.dtype)))
        self.in_names, self.out_names, self.out_avals = in_names, out_names, out_avals
        n_params, n_outs = len(in_names), len(out_avals)
        all_in = in_names + out_names + ([pname] if pname else [])

        def _body(*args):
            operands = list(args)
            if pname:
                operands.append(partition_id_tensor())
            return tuple(_bass_exec_p.bind(
                *operands, out_avals=tuple(out_avals),
                in_names=tuple(all_in), out_names=tuple(out_names),
                lowering_input_output_aliases=(),
                sim_require_finite=True, sim_require_nnan=True, nc=nc))

        devices = jax.devices()[:n_cores]
        self.mesh = Mesh(np.asarray(devices), ("core",))
        self.sh = NamedSharding(self.mesh, PartitionSpec("core"))
        self.fn = jax.jit(
            shard_map(_body, mesh=self.mesh,
                      in_specs=(PartitionSpec("core"),) * (n_params + n_outs),
                      out_specs=(PartitionSpec("core"),) * n_outs,
                      check_rep=False),
            donate_argnums=tuple(range(n_params, n_params + n_outs)),
            keep_unused=True)
        self._zs = [(n_cores * a.shape[0], *a.shape[1:]) for a in out_avals]
        self._zd = [a.dtype for a in out_avals]
        self._dev_in = None

    def stage(self, in_maps):
        ci = [np.concatenate([np.ascontiguousarray(in_maps[c][n])
                              for c in range(self.n_cores)], axis=0)
              for n in self.in_names]
        self._dev_in = [self.jax.device_put(x, self.sh) for x in ci]
        for x in self._dev_in:
            x.block_until_ready()

    def run(self):
        zeros = [self.jax.device_put(np.zeros(s, d), self.sh)
                 for s, d in zip(self._zs, self._zd)]
        outs = self.fn(*self._dev_in, *zeros)
        for o in outs:
            o.block_until_ready()
        return outs

    def results(self, outs):
        res = []
        for c in range(self.n_cores):
            d = {}
            for i, n in enumerate(self.out_names):
                a = np.asarray(outs[i]).reshape(self.n_cores, *self.out_avals[i].shape)
                d[n] = a[c]
            res.append(d)
        return res


# ----------------------------------------------------------------- kernel()

_CACHE = {}

# timing info from the last kernel() call, for test.py
last_run_info = {}


def _consts():
    iota = np.tile(np.arange(P, dtype=np.float32), (P, 1))
    ident = np.eye(P, dtype=np.float32)
    ones1 = np.ones((1, P), dtype=np.float32)
    return iota, ident, ones1


def kernel(x, edge_index, batch, num_graphs=None, W1=None, b1=None, W2=None,
           b2=None, Wf1=None, bf1=None, Wf2=None, bf2=None):
    bfnp = mybir.dt.np(BF16)
    x = np.asarray(x, dtype=np.float32)
    W1 = np.asarray(W1, dtype=np.float32)
    b1 = np.asarray(b1, dtype=np.float32).reshape(1, HID)
    W2 = np.asarray(W2, dtype=np.float32)
    b2 = np.asarray(b2, dtype=np.float32).reshape(1, HID)
    Wf1 = np.asarray(Wf1, dtype=np.float32)
    bf1 = np.asarray(bf1, dtype=np.float32).reshape(1, HID)
    Wf2 = np.asarray(Wf2, dtype=np.float32)
    bf2 = np.asarray(bf2, dtype=np.float32).reshape(1, 4)
    W1c = np.concatenate([W1, b1], axis=0)  # [3, HID] folded bias
    W2c = np.concatenate([W2, b2], axis=0)  # [65, HID]

    ei = np.asarray(edge_index)
    bt = np.asarray(batch)
    key = hash((ei.tobytes(), bt.tobytes()))
    if key not in _CACHE:
        t0 = time.time()
        struct, percore, deg, gptr, n0s, g0 = _prep(ei, bt)
        nc1 = _build_launch(struct, 1)
        r1 = _Runner(nc1)
        nc2 = _build_launch(struct, 2)
        r2 = _Runner(nc2)
        _CACHE[key] = (struct, percore, r1, r2)
        last_run_info["build_s"] = time.time() - t0
    struct, percore, r1, r2 = _CACHE[key]

    iota, ident, ones1 = _consts()

    # launch 1: table = zero-padded x in bf16 (first 2 of 128 cols),
    # rotated per core so own rows are fronted (pure index movement)
    xpad = np.zeros((N_PAD, TBLW), dtype=bfnp)
    xpad[:N_NODES, 0:2] = x.astype(bfnp)
    rots = [np.concatenate([np.arange(pc["n0"], N_PAD), np.arange(0, pc["n0"])])
            for pc in percore]

    maps1 = []
    for c in range(NCORES):
        pc = percore[c]
        maps1.append({
            "table": xpad[rots[c]], "idx": pc["idx_packed"], "dst_rel": pc["dst_rel"],
            "deg_prod": pc["deg_prod"], "deg_own": pc["deg_own"],
            "iota": iota, "ident": ident, "ones1": ones1,
            "Wc": W1c,
        })
    t0 = time.time()
    r1.stage(maps1)
    last_run_info["stage1_s"] = time.time() - t0
    t0 = time.time()
    outs1 = r1.run()
    last_run_info["run1_s"] = time.time() - t0
    res1 = r1.results(outs1)

    # host reassembly of the hidden table (pure data movement)
    h1s_full = np.zeros((N_PAD, TBLW), dtype=bfnp)
    for c in range(NCORES):
        pc = percore[c]
        n0, n1 = pc["n0"], pc["n1"]
        h1s_full[n0:n1, 0:HID] = res1[c]["h1s"][0:n1 - n0]

    maps2 = []
    for c in range(NCORES):
        pc = percore[c]
        maps2.append({
            "table": h1s_full[rots[c]], "idx": pc["idx_packed"], "dst_rel": pc["dst_rel"],
            "deg_own": pc["deg_own"], "g_rel": pc["g_rel"],
            "deg_row": np.ascontiguousarray(pc["deg_own"].T).reshape(1, -1),
            "iota": iota, "ident": ident, "ones1": ones1,
            "Wc": W2c, "Wf1": Wf1, "bf1": bf1,
            "Wf2": Wf2, "bf2": bf2,
        })
    t0 = time.time()
    r2.stage(maps2)
    last_run_info["stage2_s"] = time.time() - t0
    t0 = time.time()
    outs2 = r2.run()
    last_run_info["run2_s"] = time.time() - t0
    res2 = r2.results(outs2)

    out = np.zeros((N_GRAPHS, 4), dtype=np.float32)
    for c in range(NCORES):
        pc = percore[c]
        out[pc["g0"]:pc["g1"]] = res2[c]["out"][0:pc["g1"] - pc["g0"]]

    last_run_info["runners"] = (r1, r2)
    last_run_info["maps"] = (maps1, maps2)
    return out


def _burst_time(rx, burst: int = 6, rounds: int = 10):
    """Median amortized wall time per dispatch over bursts.

    The fastest round is dropped (pipeline-warmth outlier), then the
    median of the rest is returned — the tunnel dispatch cost is noisy
    (+-1.5ms) and min-statistics systematically underestimate."""
    import time as _t
    rx.run()  # warm
    vals = []
    for _ in range(rounds):
        t0 = _t.perf_counter()
        outs = None
        for _ in range(burst):
            zeros = [rx.jax.device_put(np.zeros(sh, d), rx.sh)
                     for sh, d in zip(rx._zs, rx._zd)]
            outs = rx.fn(*rx._dev_in, *zeros)
        for o in outs:
            o.block_until_ready()
        vals.append((_t.perf_counter() - t0) / burst)
    vals = sorted(vals)[1:]
    return sorted(vals)[len(vals) // 2]


def measure_hw_ns(reps: int = 33, reps_lo: int = 9):
    """On-device exec time per launch via work-repetition slope.

    Host dispatch through the axon tunnel has a ~12ms per-dispatch cost
    (with ~+-1.5ms noise) that has nothing to do with device execution.
    To time the device work we build two timing variants of each launch
    (identical I/O; the whole edge pass + compute repeated `reps_lo` and
    `reps` times, re-deriving identical values) and use
      t_device(per pass) = (t(reps) - t(reps_lo)) / (reps - reps_lo)
    with median burst statistics.  Both rep counts are large enough that
    the bursts are device-bound, so the dispatch constant (and any
    RPC/device pipelining) cancels in the difference; a three-point
    linearity check of this estimator agreed within ~5%.  One pass of a
    launch is the full device work of that launch minus its one-time
    constant/metadata loads (a few MB of contiguous DMA), which we add
    back as an estimate from bytes at stream rate plus drain overhead."""
    struct, percore, r1, r2 = next(iter(_CACHE.values()))
    maps1, maps2 = last_run_info["maps"]

    detail = {}
    total = 0.0
    for layer, maps in ((1, maps1), (2, maps2)):
        ts = {}
        for r in (reps_lo, reps):
            key = ("timing", layer, r)
            if key not in _CACHE:
                nct = _build_launch(struct, layer, reps=r, timing=True)
                rx = _Runner(nct)
                rx.stage(maps)
                _CACHE[key] = rx
            ts[r] = _burst_time(_CACHE[key])
        slope = (ts[reps] - ts[reps_lo]) / (reps - reps_lo)
        # one-time device work not captured by the slope: constant /
        # index / metadata loads at launch start (contiguous DMA).
        fixed_bytes = sum(np.asarray(v).nbytes for k, v in maps[0].items()
                          if k != "table")
        fixed_s = fixed_bytes / 300e9 + 20e-6
        detail[f"launch{layer}"] = {
            "slope_us": round(slope * 1e6, 1),
            "tlo_us": round(ts[reps_lo] * 1e6, 1),
            "tR_us": round(ts[reps] * 1e6, 1),
            "fixed_us": round(fixed_s * 1e6, 1),
        }
        total += max(slope, 0.0) + fixed_s
    last_run_info["hw_detail"] = detail
    return total * 1e9


# revision 35
# speedup vs baseline: 1.0238x; 1.0238x over previous
"""GCN graph classifier on 8 Trainium2 NeuronCores (Bass/Tile).

Strategy (graph/data parallel per the sharding hint):
- Nodes are split into 8 contiguous ranges aligned to graph boundaries; each
  core owns the destination side of every edge landing in its range (plus one
  self-loop edge per owned node), pooling and the MLP head for its graphs.
- Message passing is computed as agg = dinv * ((S+I) @ (dinv * h)) via
  per-edge row gathers (dma_gather, 256B bf16 rows) and one-hot matmuls that
  scatter-accumulate each 128-edge chunk into its 128-node window in PSUM.
- Two launches: layer 1 (aggregates 2-wide x, emits bf16 dinv*relu(conv1)
  rows), host reassembles the full hidden table, layer 2 + mean-pool + head.
- Tables are bf16 with 256B rows (the dma_gather element granularity), so
  gathered rows feed the scatter matmuls directly with no per-edge cast.
- Each core's table copy is rotated so its own rows are fronted: self-loop
  terms come from contiguous DMA + an identity/diag(dinv^2) matmul instead
  of random gathers, and the uniform source view lets the chunk windows be
  tuned for minimal 128-padding.
- The aggregate accumulates TRANSPOSED in PSUM (accT = msg^T @ onehot), so
  the conv matmul consumes it without a PE transpose; conv bias rides a
  1-partition matmul accumulate (sqrt(deg)-scaled for layer 2 so the
  deferred dst-side dinv activation scale lands exactly); PSUM evacuations
  ride the Activation engine; one-hot is_equal builds stay on Vector
  (TensorTensor on GpSimd is rejected by walrus; Act is per-partition-bias
  limited).
- All per-core variation (indices, one-hot selectors, degree data, rotated
  tables) is input data; the compiled program is identical across cores
  (SPMD).  Measured bottleneck: the 256B random-row gather stream itself
  (~115 GB/s/core effective); see measure_hw_ns for the timing method.
- Known-fatal: negative "ignored" trailing gather indices desync the mesh;
  identical pad rows serialize DMA (keep _PAD_SPREAD); single_packet=True
  wedges the device.

Self-contained: no imports from the problem directory.
"""
import functools
import time

import numpy as np

import concourse.bass as bass
import concourse.bacc as bacc
import concourse.mybir as mybir
import concourse.tile as tile

N_NODES = 100_000
N_PAD = 100_352            # 128-aligned, 3*32768 + 2048
N_EDGES = 1_200_000
N_GRAPHS = 512
HID = 64
TBLW = 128                 # table row width in bf16 (256B = gather elem)
NCORES = 8
P = 128
CHUNK_ROWS = 32_768        # int16-addressable table chunk (window width)
NCHUNK = 4                 # gather windows into the (rotated) table
# Edge->chunk assignment boundaries and window start rows.  Each core's table
# copy is ROTATED so its own nodes sit at rows [0, n1-n0): self-loop rows are
# then at compile-time offsets (contiguous DMA, no gather), and every core
# sees the same uniform source distribution, which lets the chunk boundaries
# be tuned for minimal 128-padding (K per (slot,chunk) piece ~ 4+4+3+3).
CH_BOUNDS = [28_928, 57_856, 79_104]
CH_STARTS = [0, 28_928, 57_856, N_PAD - CHUNK_ROWS]
BATCH_SLOTS = 4            # slots per gather batch
GMAX = 96                  # padded graphs per core (real ~64)
SENT = 30_000.0            # one-hot sentinel (never matches iota 0..127)
_PAD_SPREAD = True         # pad gather slots: spread over 2048 rows vs row 0
OH_DVE_FRAC = 1.0          # fraction of one-hot build columns on DVE (rest Pool;
                           # plain TensorTensor is rejected on Pool by walrus)
ROTATE_QUEUES = True       # balance SWDGE queues across unequal chunk calls
SINGLE_PACKET = False      # dma_gather single_packet flag

F32 = mybir.dt.float32
BF16 = mybir.dt.bfloat16
I16 = mybir.dt.int16


# ---------------------------------------------------------------- host prep

def _prep(edge_index: np.ndarray, batch: np.ndarray):
    """All index-side preprocessing (sharding metadata). No float math on
    values here - only integer index manipulation derived from the graph
    structure, plus integer degree counts (rsqrt happens on device)."""
    src = np.asarray(edge_index[0], dtype=np.int64)
    dst = np.asarray(edge_index[1], dtype=np.int64)
    batch = np.asarray(batch, dtype=np.int64)

    deg = np.bincount(dst, minlength=N_NODES) + 1  # int degree (self-loop +1)

    gptr = np.searchsorted(batch, np.arange(N_GRAPHS + 1))
    targets = (np.arange(1, NCORES) * N_NODES) // NCORES
    gsplit = np.searchsorted(gptr, targets)
    g0 = np.concatenate([[0], gsplit, [N_GRAPHS]])
    n0s = gptr[g0]  # node range starts per core (len 9)

    order = np.argsort(dst, kind="stable")
    dst_s = dst[order]
    src_s = src[order]
    e0s = np.searchsorted(dst_s, n0s)

    cores = []
    for c in range(NCORES):
        n0, n1 = int(n0s[c]), int(n0s[c + 1])
        eso = src_s[e0s[c]:e0s[c + 1]]      # original (global) source ids
        ed = dst_s[e0s[c]:e0s[c + 1]]
        # self-loop edges are NOT in the gather stream: they are served by a
        # contiguous read of the core's own (rotation-fronted) table rows
        es = (eso - n0) % N_PAD             # rotated source rows
        slot = (ed - n0) >> 7
        chunk = np.searchsorted(np.array(CH_BOUNDS), es, side="right")
        o2 = np.lexsort((slot, chunk, slot >> 3))  # (batch, chunk, slot)
        cores.append({
            "n0": n0, "n1": n1, "g0": int(g0[c]), "g1": int(g0[c + 1]),
            "es": es[o2], "eso": eso[o2], "ed": ed[o2],
            "slot": slot[o2], "chunk": chunk[o2],
            "W": int(-(-(n1 - n0) // P)),
        })

    W_SLOTS = max(cr["W"] for cr in cores)
    W_SLOTS = -(-W_SLOTS // BATCH_SLOTS) * BATCH_SLOTS  # pad to batch multiple
    NB = W_SLOTS // BATCH_SLOTS

    # per (slot, chunk) 128-block counts, cross-core max
    counts = np.zeros((NCORES, W_SLOTS, NCHUNK), dtype=np.int64)
    for c, cr in enumerate(cores):
        key = cr["slot"] * NCHUNK + cr["chunk"]
        bc = np.bincount(key, minlength=W_SLOTS * NCHUNK)
        counts[c] = bc.reshape(W_SLOTS, NCHUNK)
    K = np.maximum(-(-counts.max(axis=0) // P), 1)  # [W_SLOTS, NCHUNK] blocks

    # block layout: ordered by (batch, chunk, slot-in-batch, k)
    # block_base[s, ch] = index of first block of that piece
    block_base = np.zeros((W_SLOTS, NCHUNK), dtype=np.int64)
    call_meta = []  # per gather call: (chunk, edge_base, n_edges)
    nb_blocks = 0
    for b in range(NB):
        sl = slice(b * BATCH_SLOTS, (b + 1) * BATCH_SLOTS)
        for ch in range(NCHUNK):
            kb = K[sl, ch]
            block_base[sl, ch] = nb_blocks + np.concatenate([[0], np.cumsum(kb)[:-1]])
            ncall = int(kb.sum()) * P
            call_meta.append((ch, nb_blocks * P, ncall))
            nb_blocks += int(kb.sum())
    NSUB = nb_blocks
    NE_LAY = NSUB * P  # total gather slots per layer

    # per-sub (block) metadata: slot position + start/stop flags, slot-major
    sub_of = []  # in processing order: (sub_idx, slot, is_first, is_last)
    for b in range(NB):
        for s in range(b * BATCH_SLOTS, (b + 1) * BATCH_SLOTS):
            subs = []
            for ch in range(NCHUNK):
                for k in range(int(K[s, ch])):
                    subs.append(int(block_base[s, ch]) + k)
            for i, sub in enumerate(subs):
                sub_of.append((sub, s, i == 0, i == len(subs) - 1))

    # per-core data arrays
    percore = []
    for c, cr in enumerate(cores):
        es, ed, slot, chunk = cr["es"], cr["ed"], cr["slot"], cr["chunk"]
        eso = cr["eso"]
        key = slot * NCHUNK + chunk
        grp_order = np.lexsort((np.arange(len(es)), key))  # already sorted but safe
        # rank within (slot, chunk) group
        sort_key = key[grp_order]
        ranks = np.arange(len(es)) - np.searchsorted(sort_key, sort_key)
        # position of each edge
        pos = block_base[slot[grp_order], chunk[grp_order]] * P + ranks
        esg = es[grp_order]
        esog = eso[grp_order]
        edg = ed[grp_order]

        # pad slots gather garbage rows (masked by the one-hot sentinel);
        # spread them across each chunk's rows to avoid hammering one HBM
        # row with thousands of identical descriptors
        rng_pad = np.random.default_rng(12345)
        if _PAD_SPREAD:
            # spread pad descriptors across each call's whole chunk (the last
            # chunk is short); identical/clustered pad rows serialize the DMA
            # engines on HBM row conflicts
            idx_flat = np.empty(NE_LAY, dtype=np.int16)
            for ch_, ebase_, ncall_ in call_meta:
                idx_flat[ebase_:ebase_ + ncall_] = rng_pad.integers(
                    0, CHUNK_ROWS, size=ncall_).astype(np.int16)
        else:
            idx_flat = np.zeros(NE_LAY, dtype=np.int16)
        idx_flat[pos] = (esg - np.array(CH_STARTS)[chunk[grp_order]]).astype(np.int16)

        dst_rel = np.full((P, NSUB), SENT, dtype=np.float32)
        dst_rel[pos % P, pos >> 7] = (edg - cr["n0"] - slot[grp_order] * P).astype(np.float32)
        # slot-major column order so each slot's blocks are contiguous (the
        # batched one-hot build slices a contiguous range per slot)
        dst_rel = dst_rel[:, [so[0] for so in sub_of]]
        dst_rel = dst_rel.astype(mybir.dt.np(BF16))  # exact for 0..127 + sentinel

        deg_prod = np.ones((P, NSUB), dtype=np.float32)
        deg_prod[pos % P, pos >> 7] = (deg[esog] * deg[edg]).astype(np.float32)

        nown = cr["n1"] - cr["n0"]
        ar = np.arange(nown)
        deg_own = np.ones((P, W_SLOTS), dtype=np.float32)
        deg_own[ar % P, ar >> 7] = deg[cr["n0"]:cr["n1"]].astype(np.float32)

        g_rel = np.full((P, W_SLOTS), SENT, dtype=np.float32)
        g_rel[ar % P, ar >> 7] = (batch[cr["n0"]:cr["n1"]] - cr["g0"]).astype(np.float32)
        g_rel = g_rel.astype(mybir.dt.np(BF16))  # exact (values < 256 + sentinel)

        # pack idx into [128, NE_LAY//16] int16 col-major-16 replicated
        cols = NE_LAY // 16
        arr = np.zeros((16, cols), dtype=np.int16)
        j = np.arange(NE_LAY)
        arr[j % 16, j // 16] = idx_flat
        idx_packed = np.tile(arr, (8, 1))

        percore.append({
            **{k: cr[k] for k in ("n0", "n1", "g0", "g1", "W")},
            "idx_packed": idx_packed, "dst_rel": dst_rel,
            "deg_prod": deg_prod, "deg_own": deg_own, "g_rel": g_rel,
        })

    struct = {
        "W_SLOTS": W_SLOTS, "NB": NB, "NSUB": NSUB, "NE_LAY": NE_LAY,
        "K": K, "block_base": block_base, "call_meta": call_meta,
        "sub_of": sub_of,
    }
    return struct, percore, deg, gptr, n0s, g0


# ------------------------------------------------------------- bass program

def _build_launch(struct, layer: int, reps: int = 1, timing: bool = False, part: str = 'all'):
    """Build the SPMD Bass program for layer 1 or layer 2(+pool+mlp)."""
    W_SLOTS, NB = struct["W_SLOTS"], struct["NB"]
    NSUB, NE_LAY = struct["NSUB"], struct["NE_LAY"]
    K, block_base = struct["K"], struct["block_base"]
    call_meta, sub_of = struct["call_meta"], struct["sub_of"]

    nc = bacc.Bacc("TRN2", num_swdge_queues=4)
    table = nc.dram_tensor("table", (N_PAD, TBLW), BF16, kind="ExternalInput")
    idx_in = nc.dram_tensor("idx", (P, NE_LAY // 16), I16, kind="ExternalInput")
    dst_rel_in = nc.dram_tensor("dst_rel", (P, NSUB), BF16, kind="ExternalInput")
    deg_own_in = nc.dram_tensor("deg_own", (P, W_SLOTS), F32, kind="ExternalInput")
    iota_in = nc.dram_tensor("iota", (P, P), F32, kind="ExternalInput")
    ones_in = nc.dram_tensor("ones1", (1, P), F32, kind="ExternalInput")
    ident_in = nc.dram_tensor("ident", (P, P), F32, kind="ExternalInput")
    msg_w = 2 if layer == 1 else HID
    # conv weight with the bias folded in as a final row ([W; b])
    w_in = nc.dram_tensor("Wc", (msg_w + 1, HID), F32, kind="ExternalInput")
    if layer == 1:
        deg_prod_in = nc.dram_tensor("deg_prod", (P, NSUB), F32, kind="ExternalInput")
        if timing:
            out_t = nc.dram_tensor("h1s_scratch", (W_SLOTS * P, HID), BF16)
            dummy_t = nc.dram_tensor("tdummy0", (1, 4), F32, kind="ExternalOutput")
        else:
            out_t = nc.dram_tensor("h1s", (W_SLOTS * P, HID), BF16, kind="ExternalOutput")
    else:
        g_rel_in = nc.dram_tensor("g_rel", (P, W_SLOTS), BF16, kind="ExternalInput")
        deg_row_in = nc.dram_tensor("deg_row", (1, W_SLOTS * P), F32, kind="ExternalInput")
        wf1_in = nc.dram_tensor("Wf1", (HID, HID), F32, kind="ExternalInput")
        bf1_in = nc.dram_tensor("bf1", (1, HID), F32, kind="ExternalInput")
        wf2_in = nc.dram_tensor("Wf2", (HID, 4), F32, kind="ExternalInput")
        bf2_in = nc.dram_tensor("bf2", (1, 4), F32, kind="ExternalInput")
        if timing:
            out_t = nc.dram_tensor("out_scratch", (GMAX, 4), F32)
            dummy_t = nc.dram_tensor("tdummy0", (1, 4), F32, kind="ExternalOutput")
        else:
            out_t = nc.dram_tensor("out", (GMAX, 4), F32, kind="ExternalOutput")

    # organize subs per slot for slot-major processing
    slot_subs = [[] for _ in range(W_SLOTS)]
    for sub, s, first, last in sub_of:
        slot_subs[s].append(sub)
    # slot-major column starts into the (permuted) dst_rel array
    slot_col0 = np.zeros(W_SLOTS, dtype=np.int64)
    acc_cols = 0
    for s in range(W_SLOTS):
        slot_col0[s] = acc_cols
        acc_cols += len(slot_subs[s])

    # sub -> (call index, block-within-call) for gather tile slicing
    sub_call = np.zeros(NSUB, dtype=np.int64)
    sub_kloc = np.zeros(NSUB, dtype=np.int64)
    for ci, (ch, ebase, ncall) in enumerate(call_meta):
        b0 = ebase // P
        nb = ncall // P
        sub_call[b0:b0 + nb] = ci
        sub_kloc[b0:b0 + nb] = np.arange(nb)

    with tile.TileContext(nc) as tc:
        with tc.tile_pool(name="const", bufs=1) as cpool, \
             tc.tile_pool(name="meta", bufs=1) as mpool, \
             tc.tile_pool(name="gat", bufs=5) as gpool, \
             tc.tile_pool(name="own", bufs=3) as opool, \
             tc.tile_pool(name="gbf", bufs=8) as gbpool, \
             tc.tile_pool(name="work", bufs=3) as wpool, \
             tc.tile_pool(name="oh", bufs=3) as ohpool, \
             tc.tile_pool(name="pacc", bufs=3, space="PSUM") as pacc, \
             tc.tile_pool(name="ptp", bufs=2, space="PSUM") as ptp, \
             tc.tile_pool(name="ppool", bufs=1, space="PSUM") as ppool:

            # ---- load constants / metadata
            iota_t = cpool.tile([P, P], F32)
            nc.sync.dma_start(out=iota_t[:], in_=iota_in[:])
            iota_bf = cpool.tile([P, P], BF16)
            nc.vector.tensor_copy(out=iota_bf[:], in_=iota_t[:])
            ident_t = cpool.tile([P, P], F32)
            nc.sync.dma_start(out=ident_t[:], in_=ident_in[:])
            ident_bf = cpool.tile([P, P], BF16)
            nc.vector.tensor_copy(out=ident_bf[:], in_=ident_t[:])
            ones_t = cpool.tile([1, P], F32)
            nc.sync.dma_start(out=ones_t[:], in_=ones_in[:])
            w_t = cpool.tile([msg_w + 1, HID], F32)
            nc.sync.dma_start(out=w_t[:], in_=w_in[:])
            b_row_t = cpool.tile([1, HID], F32)
            nc.sync.dma_start(out=b_row_t[:], in_=w_in[msg_w:msg_w + 1, :])

            idx_t = mpool.tile([P, NE_LAY // 16], I16)
            nc.sync.dma_start(out=idx_t[:], in_=idx_in[:])
            dst_rel_t = mpool.tile([P, NSUB], BF16)
            nc.sync.dma_start(out=dst_rel_t[:], in_=dst_rel_in[:])
            deg_own_t = mpool.tile([P, W_SLOTS], F32)
            nc.sync.dma_start(out=deg_own_t[:], in_=deg_own_in[:])

            # dinv2 = 1/deg (self-loop weight), dinv = rsqrt(deg)
            dinv2_own_t = mpool.tile([P, W_SLOTS], F32)
            nc.vector.reciprocal(out=dinv2_own_t[:], in_=deg_own_t[:])
            dinv_own_t = deg_own_t
            nc.scalar.sqrt(out=dinv_own_t[:], in_=dinv2_own_t[:])

            if layer == 1:
                # per-edge norm = rsqrt(deg[src] * deg[dst]), in place
                deg_prod_t = mpool.tile([P, NSUB], F32)
                nc.sync.dma_start(out=deg_prod_t[:], in_=deg_prod_in[:])
                norm_t = deg_prod_t
                nc.vector.reciprocal(out=norm_t[:], in_=deg_prod_t[:])
                nc.scalar.sqrt(out=norm_t[:], in_=norm_t[:])
                # per-slot diag(dinv^2) for the self-loop matmul (bf16)
                diag2_all = cpool.tile([P, W_SLOTS, P], BF16)
                nc.vector.tensor_tensor(
                    out=diag2_all[:],
                    in0=ident_bf[:].rearrange("p (k c) -> p k c", k=1)
                        .broadcast_to((P, W_SLOTS, P)),
                    in1=dinv2_own_t[:].rearrange("p (k o) -> p k o", o=1)
                        .broadcast_to((P, W_SLOTS, P)),
                    op=mybir.AluOpType.mult)

            if layer == 2:
                # per-slot sqrt(deg) rows for the bias matmul (bias must not
                # be scaled by the deferred dinv: h = relu(dinv*(agg@W) + b)
                # is realized as relu(dinv*(agg@W + sqrtdeg*b)))
                deg_row_t = mpool.tile([1, W_SLOTS * P], F32)
                nc.sync.dma_start(out=deg_row_t[:], in_=deg_row_in[:])
                sqdeg_row_t = deg_row_t
                nc.scalar.sqrt(out=sqdeg_row_t[:], in_=deg_row_t[:])

            if layer == 2:
                g_rel_t = mpool.tile([P, W_SLOTS], BF16)
                nc.sync.dma_start(out=g_rel_t[:], in_=g_rel_in[:])
                wf1_t = cpool.tile([HID, HID], F32)
                nc.sync.dma_start(out=wf1_t[:], in_=wf1_in[:])
                wf2_t = cpool.tile([HID, 4], F32)
                nc.sync.dma_start(out=wf2_t[:], in_=wf2_in[:])
                bf1_t = cpool.tile([1, HID], F32)
                nc.sync.dma_start(out=bf1_t[:], in_=bf1_in[:])
                bf2_t = cpool.tile([1, 4], F32)
                nc.sync.dma_start(out=bf2_t[:], in_=bf2_in[:])
                # head bias broadcasts via ones-matmul
                bb2_ps = ptp.tile([P, HID], F32, space="PSUM", tag="hps")
                nc.tensor.matmul(out=bb2_ps[:], lhsT=ones_t[:], rhs=bf1_t[:],
                                 start=True, stop=True)
                bf1_bcast = cpool.tile([P, HID], F32)
                nc.vector.tensor_copy(out=bf1_bcast[:], in_=bb2_ps[:])
                bb3_ps = ptp.tile([P, 4], F32, space="PSUM", tag="hps")
                nc.tensor.matmul(out=bb3_ps[:], lhsT=ones_t[:], rhs=bf2_t[:],
                                 start=True, stop=True)
                bf2_bcast = cpool.tile([P, 4], F32)
                nc.vector.tensor_copy(out=bf2_bcast[:], in_=bb3_ps[:])
                pool_ps = ppool.tile([GMAX, HID + 1], F32, space="PSUM")

            # ---- main loop over batches (reps>1 repeats the whole
            # edge pass for timing-slope measurement; outputs stay valid
            # because each rep re-derives the same values).  part='gather'
            # instead repeats only the dma_gather calls (WAW-chained on the
            # same tile); part='compute' repeats only the scatter/compute.
            ncall_per_b = NCHUNK
            grep = reps if part == "gather" else 1
            crep = reps if part == "compute" else 1
            arep = reps if part == "all" else 1
            for rep, b in [(r, b) for r in range(arep) for b in range(NB)]:
                gtiles = []
                for ci in range(b * ncall_per_b, (b + 1) * ncall_per_b):
                    ch, ebase, ncall = call_meta[ci]
                    g_t = gpool.tile([P, ncall // P, TBLW], BF16, tag=f"g{ci % ncall_per_b}")
                    qn = (ci + ci // 4) % 4 if ROTATE_QUEUES else ci % 4
                    for _ in range(grep):
                        nc.gpsimd.dma_gather(
                            out_ap=g_t[:],
                            in_ap=table[CH_STARTS[ch]: CH_STARTS[ch] + CHUNK_ROWS, :],
                            idxs_ap=idx_t[:, ebase // 16:(ebase + ncall) // 16],
                            num_idxs=ncall, num_idxs_reg=ncall, elem_size=TBLW,
                            single_packet=SINGLE_PACKET, queue_num=qn)
                    if layer == 1:
                        # fuse per-edge norm into the (tiny 2-wide) messages
                        b0 = ebase // P
                        g_bf = gbpool.tile([P, ncall // P, msg_w], BF16, tag="gb")
                        nc.vector.tensor_tensor(
                            out=g_bf[:], in0=g_t[:, :, 0:msg_w],
                            in1=norm_t[:, b0:b0 + ncall // P]
                                .rearrange("p (k o) -> p k o", o=1)
                                .broadcast_to((P, ncall // P, msg_w)),
                            op=mybir.AluOpType.mult)
                        gtiles.append(g_bf)
                    else:
                        gtiles.append(g_t)

                if part == "gather":
                    continue
                if layer == 2:
                    goh_big = ohpool.tile([P, BATCH_SLOTS, GMAX], BF16, tag="goh")
                    nc.vector.tensor_tensor(
                        out=goh_big[:],
                        in0=iota_bf[:, 0:GMAX]
                            .rearrange("p (k c) -> p k c", k=1)
                            .broadcast_to((P, BATCH_SLOTS, GMAX)),
                        in1=g_rel_t[:, b * BATCH_SLOTS:(b + 1) * BATCH_SLOTS]
                            .rearrange("p (k o) -> p k o", o=1)
                            .broadcast_to((P, BATCH_SLOTS, GMAX)),
                        op=mybir.AluOpType.is_equal)
                for crep_i, s in [(r, s) for r in range(crep)
                                  for s in range(b * BATCH_SLOTS, (b + 1) * BATCH_SLOTS)]:
                    subs = slot_subs[s]
                    k = len(subs)
                    col0 = slot_col0[s]
                    # accumulate the TRANSPOSED aggregate: accT[f, d] so the
                    # conv matmul consumes it directly (no PE transpose)
                    acc = pacc.tile([msg_w, P], F32, space="PSUM", tag="acc")
                    # batched 0/1 one-hots for all this slot's blocks at once
                    oh_big = ohpool.tile([P, k, P], BF16, tag="oh")
                    nc.vector.tensor_tensor(
                        out=oh_big[:],
                        in0=iota_bf[:]
                            .rearrange("p (k c) -> p k c", k=1)
                            .broadcast_to((P, k, P)),
                        in1=dst_rel_t[:, col0:col0 + k]
                            .rearrange("p (k o) -> p k o", o=1)
                            .broadcast_to((P, k, P)),
                        op=mybir.AluOpType.is_equal)
                    for i, sub in enumerate(subs):
                        ci = int(sub_call[sub])
                        kloc = int(sub_kloc[sub])
                        g_t = gtiles[ci % ncall_per_b]
                        lhs = g_t[:, kloc, :] if layer == 1 else g_t[:, kloc, 0:HID]
                        nc.tensor.matmul(
                            out=acc[:], lhsT=lhs,
                            rhs=oh_big[:, i, :],
                            start=(i == 0), stop=False)
                    # self-loop term: this core's own rows are rotation-
                    # fronted at compile-time offsets -> contiguous DMA plus
                    # an identity (L2) / diag(dinv^2) (L1) matmul
                    own_t = opool.tile([P, TBLW], BF16, tag="own")
                    nc.sync.dma_start(out=own_t[:], in_=table[s * P:(s + 1) * P, :])
                    rhs_self = diag2_all[:, s, :] if layer == 1 else ident_bf[:]
                    nc.tensor.matmul(out=acc[:], lhsT=own_t[:, 0:msg_w],
                                     rhs=rhs_self, start=False, stop=True)

                    # ---- slot tail: evacuate accT, conv matmul + bias row
                    a2t = wpool.tile([msg_w, P], F32, tag="a2t")
                    nc.scalar.copy(out=a2t[:], in_=acc[:])
                    h_ps = ptp.tile([P, HID], F32, space="PSUM", tag="hps")
                    nc.tensor.matmul(out=h_ps[:], lhsT=a2t[:], rhs=w_t[0:msg_w, :],
                                     start=True, stop=False)
                    if layer == 1:
                        nc.tensor.matmul(out=h_ps[:], lhsT=ones_t[:],
                                         rhs=b_row_t[:],
                                         start=False, stop=True)
                        h1s = wpool.tile([P, HID], BF16, tag="h1s")
                        # emit dinv[node] * relu(conv1): dinv_src prefold for
                        # the layer-2 gather (bf16 table rows)
                        nc.scalar.activation(
                            out=h1s[:], in_=h_ps[:],
                            func=mybir.ActivationFunctionType.Relu,
                            scale=dinv_own_t[:, s:s + 1])
                        nc.sync.dma_start(out=out_t[s * P:(s + 1) * P, :], in_=h1s[:])
                    else:
                        # bias row scaled by sqrt(deg) so the dinv activation
                        # scale yields relu(dinv*(agg@W) + b)
                        nc.tensor.matmul(out=h_ps[:],
                                         lhsT=sqdeg_row_t[:, s * P:(s + 1) * P],
                                         rhs=b_row_t[:],
                                         start=False, stop=True)
                        h2 = wpool.tile([P, HID + 1], BF16, tag="h2")
                        nc.scalar.activation(
                            out=h2[:, 0:HID], in_=h_ps[:],
                            func=mybir.ActivationFunctionType.Relu,
                            scale=dinv_own_t[:, s:s + 1])
                        nc.vector.memset(h2[:, HID:HID + 1], 1.0)
                        nc.tensor.matmul(out=pool_ps[:],
                                         lhsT=goh_big[:, s - b * BATCH_SLOTS, :],
                                         rhs=h2[:],
                                         start=(s == 0), stop=(s == W_SLOTS - 1))

            if timing:
                d = wpool.tile([1, 4], F32, tag="dmy")
                nc.vector.memset(d[:], 0.0)
                nc.sync.dma_start(out=dummy_t[:], in_=d[:])

            # ---- pool + MLP head (layer 2)
            if layer == 2 and part not in ("gather",):
                pool_sb = wpool.tile([GMAX, HID + 1], F32, tag="pool")
                nc.vector.tensor_copy(out=pool_sb[:], in_=pool_ps[:])
                cnt = wpool.tile([GMAX, 1], F32, tag="cnt")
                nc.vector.tensor_scalar(
                    out=cnt[:], in0=pool_sb[:, HID:HID + 1], scalar1=1.0,
                    scalar2=None, op0=mybir.AluOpType.max)
                rcnt = wpool.tile([GMAX, 1], F32, tag="rcnt")
                nc.vector.reciprocal(out=rcnt[:], in_=cnt[:])
                means = wpool.tile([GMAX, HID], F32, tag="means")
                nc.scalar.mul(means[:], pool_sb[:, 0:HID], rcnt[:])
                mt_ps = ptp.tile([HID, GMAX], F32, space="PSUM", tag="tp")
                nc.tensor.transpose(out=mt_ps[:], in_=means[:],
                                    identity=ident_t[0:GMAX, 0:GMAX])
                mt = wpool.tile([HID, GMAX], F32, tag="mt")
                nc.vector.tensor_copy(out=mt[:], in_=mt_ps[:])
                f1_ps = ptp.tile([GMAX, HID], F32, space="PSUM", tag="hps")
                nc.tensor.matmul(out=f1_ps[:], lhsT=mt[:], rhs=wf1_t[:],
                                 start=True, stop=True)
                f1 = wpool.tile([GMAX, HID], F32, tag="f1")
                nc.vector.tensor_tensor(out=f1[:], in0=f1_ps[:],
                                        in1=bf1_bcast[0:GMAX, :],
                                        op=mybir.AluOpType.add)
                nc.scalar.activation(out=f1[:], in_=f1[:],
                                     func=mybir.ActivationFunctionType.Relu)
                f1t_ps = ptp.tile([HID, GMAX], F32, space="PSUM", tag="tp")
                nc.tensor.transpose(out=f1t_ps[:], in_=f1[:],
                                    identity=ident_t[0:GMAX, 0:GMAX])
                f1t = wpool.tile([HID, GMAX], F32, tag="f1t")
                nc.vector.tensor_copy(out=f1t[:], in_=f1t_ps[:])
                o_ps = ptp.tile([GMAX, 4], F32, space="PSUM", tag="hps")
                nc.tensor.matmul(out=o_ps[:], lhsT=f1t[:], rhs=wf2_t[:],
                                 start=True, stop=True)
                o_sb = wpool.tile([GMAX, 4], F32, tag="osb")
                nc.vector.tensor_tensor(out=o_sb[:], in0=o_ps[:],
                                        in1=bf2_bcast[0:GMAX, :],
                                        op=mybir.AluOpType.add)
                nc.sync.dma_start(out=out_t[:], in_=o_sb[:])

    nc.finalize()
    return nc


# ---------------------------------------------------------------- pjrt run

class _Runner:
    def __init__(self, nc, n_cores: int = NCORES):
        import jax
        from jax.sharding import Mesh, NamedSharding, PartitionSpec
        from jax.experimental.shard_map import shard_map
        from concourse.bass2jax import (
            _bass_exec_p, install_neuronx_cc_hook, partition_id_tensor)

        install_neuronx_cc_hook()
        self.jax = jax
        self.n_cores = n_cores
        in_names, out_names, out_avals = [], [], []
        pname = nc.partition_id_tensor.name if nc.partition_id_tensor else None
        for alloc in nc.m.functions[0].allocations:
            if not isinstance(alloc, mybir.MemoryLocationSet):
                continue
            name = alloc.memorylocations[0].name
            if alloc.kind == "ExternalInput":
                if name != pname:
                    in_names.append(name)
            elif alloc.kind == "ExternalOutput":
                out_names.append(name)
                out_avals.append(jax.core.ShapedArray(
                    tuple(alloc.tensor_shape), mybir.dt.np(alloc.dtype)))
        self.in_names, self.out_names, self.out_avals = in_names, out_names, out_avals
        n_params, n_outs = len(in_names), len(out_avals)
        all_in = in_names + out_names + ([pname] if pname else [])

        def _body(*args):
            operands = list(args)
            if pname:
                operands.append(partition_id_tensor())
            return tuple(_bass_exec_p.bind(
                *operands, out_avals=tuple(out_avals),
                in_names=tuple(all_in), out_names=tuple(out_names),
                lowering_input_output_aliases=(),
                sim_require_finite=True, sim_require_nnan=True, nc=nc))

        devices = jax.devices()[:n_cores]
        self.mesh = Mesh(np.asarray(devices), ("core",))
        self.sh = NamedSharding(self.mesh, PartitionSpec("core"))
        self.fn = jax.jit(
            shard_map(_body, mesh=self.mesh,
                      in_specs=(PartitionSpec("core"),) * (n_params + n_outs),
                      out_specs=(PartitionSpec("core"),) * n_outs,
                      check_rep=False),
            donate_argnums=tuple(range(n_params, n_params + n_outs)),
            keep_unused=True)
        self._zs = [(n_cores * a.shape[0], *a.shape[1:]) for a in out_avals]
        self._zd = [a.dtype for a in out_avals]
        self._dev_in = None

    def stage(self, in_maps):
        ci = [np.concatenate([np.ascontiguousarray(in_maps[c][n])
                              for c in range(self.n_cores)], axis=0)
              for n in self.in_names]
        self._dev_in = [self.jax.device_put(x, self.sh) for x in ci]
        for x in self._dev_in:
            x.block_until_ready()

    def run(self):
        zeros = [self.jax.device_put(np.zeros(s, d), self.sh)
                 for s, d in zip(self._zs, self._zd)]
        outs = self.fn(*self._dev_in, *zeros)
        for o in outs:
            o.block_until_ready()
        return outs

    def results(self, outs):
        res = []
        for c in range(self.n_cores):
            d = {}
            for i, n in enumerate(self.out_names):
                a = np.asarray(outs[i]).reshape(self.n_cores, *self.out_avals[i].shape)
                d[n] = a[c]
            res.append(d)
        return res


# ----------------------------------------------------------------- kernel()

_CACHE = {}

# timing info from the last kernel() call, for test.py
last_run_info = {}


def _consts():
    iota = np.tile(np.arange(P, dtype=np.float32), (P, 1))
    ident = np.eye(P, dtype=np.float32)
    ones1 = np.ones((1, P), dtype=np.float32)
    return iota, ident, ones1


def kernel(x, edge_index, batch, num_graphs=None, W1=None, b1=None, W2=None,
           b2=None, Wf1=None, bf1=None, Wf2=None, bf2=None):
    bfnp = mybir.dt.np(BF16)
    x = np.asarray(x, dtype=np.float32)
    W1 = np.asarray(W1, dtype=np.float32)
    b1 = np.asarray(b1, dtype=np.float32).reshape(1, HID)
    W2 = np.asarray(W2, dtype=np.float32)
    b2 = np.asarray(b2, dtype=np.float32).reshape(1, HID)
    Wf1 = np.asarray(Wf1, dtype=np.float32)
    bf1 = np.asarray(bf1, dtype=np.float32).reshape(1, HID)
    Wf2 = np.asarray(Wf2, dtype=np.float32)
    bf2 = np.asarray(bf2, dtype=np.float32).reshape(1, 4)
    W1c = np.concatenate([W1, b1], axis=0)  # [3, HID] folded bias
    W2c = np.concatenate([W2, b2], axis=0)  # [65, HID]

    ei = np.asarray(edge_index)
    bt = np.asarray(batch)
    key = hash((ei.tobytes(), bt.tobytes()))
    if key not in _CACHE:
        t0 = time.time()
        struct, percore, deg, gptr, n0s, g0 = _prep(ei, bt)
        nc1 = _build_launch(struct, 1)
        r1 = _Runner(nc1)
        nc2 = _build_launch(struct, 2)
        r2 = _Runner(nc2)
        _CACHE[key] = (struct, percore, r1, r2)
        last_run_info["build_s"] = time.time() - t0
    struct, percore, r1, r2 = _CACHE[key]

    iota, ident, ones1 = _consts()

    # launch 1: table = zero-padded x in bf16 (first 2 of 128 cols),
    # rotated per core so own rows are fronted (pure index movement)
    xpad = np.zeros((N_PAD, TBLW), dtype=bfnp)
    xpad[:N_NODES, 0:2] = x.astype(bfnp)
    rots = [np.concatenate([np.arange(pc["n0"], N_PAD), np.arange(0, pc["n0"])])
            for pc in percore]

    maps1 = []
    for c in range(NCORES):
        pc = percore[c]
        maps1.append({
            "table": xpad[rots[c]], "idx": pc["idx_packed"], "dst_rel": pc["dst_rel"],
            "deg_prod": pc["deg_prod"], "deg_own": pc["deg_own"],
            "iota": iota, "ident": ident, "ones1": ones1,
            "Wc": W1c,
        })
    t0 = time.time()
    r1.stage(maps1)
    last_run_info["stage1_s"] = time.time() - t0
    t0 = time.time()
    outs1 = r1.run()
    last_run_info["run1_s"] = time.time() - t0
    res1 = r1.results(outs1)

    # host reassembly of the hidden table (pure data movement)
    h1s_full = np.zeros((N_PAD, TBLW), dtype=bfnp)
    for c in range(NCORES):
        pc = percore[c]
        n0, n1 = pc["n0"], pc["n1"]
        h1s_full[n0:n1, 0:HID] = res1[c]["h1s"][0:n1 - n0]

    maps2 = []
    for c in range(NCORES):
        pc = percore[c]
        maps2.append({
            "table": h1s_full[rots[c]], "idx": pc["idx_packed"], "dst_rel": pc["dst_rel"],
            "deg_own": pc["deg_own"], "g_rel": pc["g_rel"],
            "deg_row": np.ascontiguousarray(pc["deg_own"].T).reshape(1, -1),
            "iota": iota, "ident": ident, "ones1": ones1,
            "Wc": W2c, "Wf1": Wf1, "bf1": bf1,
            "Wf2": Wf2, "bf2": bf2,
        })
    t0 = time.time()
    r2.stage(maps2)
    last_run_info["stage2_s"] = time.time() - t0
    t0 = time.time()
    outs2 = r2.run()
    last_run_info["run2_s"] = time.time() - t0
    res2 = r2.results(outs2)

    out = np.zeros((N_GRAPHS, 4), dtype=np.float32)
    for c in range(NCORES):
        pc = percore[c]
        out[pc["g0"]:pc["g1"]] = res2[c]["out"][0:pc["g1"] - pc["g0"]]

    last_run_info["runners"] = (r1, r2)
    last_run_info["maps"] = (maps1, maps2)
    return out


def _burst_time(rx, burst: int = 6, rounds: int = 10):
    """Median amortized wall time per dispatch over bursts.

    The fastest round is dropped (pipeline-warmth outlier), then the
    median of the rest is returned — the tunnel dispatch cost is noisy
    (+-1.5ms) and min-statistics systematically underestimate."""
    import time as _t
    rx.run()  # warm
    vals = []
    for _ in range(rounds):
        t0 = _t.perf_counter()
        outs = None
        for _ in range(burst):
            zeros = [rx.jax.device_put(np.zeros(sh, d), rx.sh)
                     for sh, d in zip(rx._zs, rx._zd)]
            outs = rx.fn(*rx._dev_in, *zeros)
        for o in outs:
            o.block_until_ready()
        vals.append((_t.perf_counter() - t0) / burst)
    vals = sorted(vals)[1:]
    return sorted(vals)[len(vals) // 2]


def measure_hw_ns(reps: int = 33, reps_lo: int = 9):
    """On-device exec time per launch via work-repetition slope.

    Host dispatch through the axon tunnel has a ~12ms per-dispatch cost
    (with ~+-1.5ms noise) that has nothing to do with device execution.
    To time the device work we build two timing variants of each launch
    (identical I/O; the whole edge pass + compute repeated `reps_lo` and
    `reps` times, re-deriving identical values) and use
      t_device(per pass) = (t(reps) - t(reps_lo)) / (reps - reps_lo)
    with median burst statistics.  Both rep counts are large enough that
    the bursts are device-bound, so the dispatch constant (and any
    RPC/device pipelining) cancels in the difference; a three-point
    linearity check of this estimator agreed within ~5%.  One pass of a
    launch is the full device work of that launch minus its one-time
    constant/metadata loads (a few MB of contiguous DMA), which we add
    back as an estimate from bytes at stream rate plus drain overhead."""
    struct, percore, r1, r2 = next(iter(_CACHE.values()))
    maps1, maps2 = last_run_info["maps"]

    detail = {}
    total = 0.0
    for layer, maps in ((1, maps1), (2, maps2)):
        ts = {}
        for r in (reps_lo, reps):
            key = ("timing", layer, r)
            if key not in _CACHE:
                nct = _build_launch(struct, layer, reps=r, timing=True)
                rx = _Runner(nct)
                rx.stage(maps)
                _CACHE[key] = rx
            ts[r] = _burst_time(_CACHE[key])
        slope = (ts[reps] - ts[reps_lo]) / (reps - reps_lo)
        # one-time device work not captured by the slope: constant /
        # index / metadata loads at launch start (contiguous DMA).
        fixed_bytes = sum(np.asarray(v).nbytes for k, v in maps[0].items()
                          if k != "table")
        fixed_s = fixed_bytes / 300e9 + 20e-6
        detail[f"launch{layer}"] = {
            "slope_us": round(slope * 1e6, 1),
            "tlo_us": round(ts[reps_lo] * 1e6, 1),
            "tR_us": round(ts[reps] * 1e6, 1),
            "fixed_us": round(fixed_s * 1e6, 1),
        }
        total += max(slope, 0.0) + fixed_s
    last_run_info["hw_detail"] = detail
    return total * 1e9


# revision 36
# speedup vs baseline: 1.0303x; 1.0063x over previous
"""GCN graph classifier on 8 Trainium2 NeuronCores (Bass/Tile).

Strategy (graph/data parallel per the sharding hint):
- Nodes are split into 8 contiguous ranges aligned to graph boundaries; each
  core owns the destination side of every edge landing in its range (plus one
  self-loop edge per owned node), pooling and the MLP head for its graphs.
- Message passing is computed as agg = dinv * ((S+I) @ (dinv * h)) via
  per-edge row gathers (dma_gather, 256B bf16 rows) and one-hot matmuls that
  scatter-accumulate each 128-edge chunk into its 128-node window in PSUM.
- Two launches: layer 1 (aggregates 2-wide x, emits bf16 dinv*relu(conv1)
  rows), host reassembles the full hidden table, layer 2 + mean-pool + head.
- Tables are bf16 with 256B rows (the dma_gather element granularity), so
  gathered rows feed the scatter matmuls directly with no per-edge cast.
- Each core's table copy is rotated so its own rows are fronted: self-loop
  terms come from contiguous DMA + an identity/diag(dinv^2) matmul instead
  of random gathers, and the uniform source view lets the chunk windows be
  tuned for minimal 128-padding.
- The aggregate accumulates TRANSPOSED in PSUM (accT = msg^T @ onehot), so
  the conv matmul consumes it without a PE transpose; conv bias rides a
  1-partition matmul accumulate (sqrt(deg)-scaled for layer 2 so the
  deferred dst-side dinv activation scale lands exactly); PSUM evacuations
  ride the Activation engine; one-hot is_equal builds stay on Vector
  (TensorTensor on GpSimd is rejected by walrus; Act is per-partition-bias
  limited).
- All per-core variation (indices, one-hot selectors, degree data, rotated
  tables) is input data; the compiled program is identical across cores
  (SPMD).  Measured bottleneck: the 256B random-row gather stream itself
  (~115 GB/s/core effective); see measure_hw_ns for the timing method.
- Known-fatal: negative "ignored" trailing gather indices desync the mesh;
  identical pad rows serialize DMA (keep _PAD_SPREAD); single_packet=True
  wedges the device.

Self-contained: no imports from the problem directory.
"""
import functools
import time

import numpy as np

import concourse.bass as bass
import concourse.bacc as bacc
import concourse.mybir as mybir
import concourse.tile as tile

N_NODES = 100_000
N_PAD = 100_352            # 128-aligned, 3*32768 + 2048
N_EDGES = 1_200_000
N_GRAPHS = 512
HID = 64
TBLW = 128                 # table row width in bf16 (256B = gather elem)
NCORES = 8
P = 128
CHUNK_ROWS = 32_768        # int16-addressable table chunk (window width)
NCHUNK = 4                 # gather windows into the (rotated) table
# Edge->chunk assignment boundaries and window start rows.  Each core's table
# copy is ROTATED so its own nodes sit at rows [0, n1-n0): self-loop rows are
# then at compile-time offsets (contiguous DMA, no gather), and every core
# sees the same uniform source distribution, which lets the chunk boundaries
# be tuned for minimal 128-padding (K per (slot,chunk) piece ~ 4+4+3+3).
CH_BOUNDS = [28_928, 57_856, 79_104]
CH_STARTS = [0, 28_928, 57_856, N_PAD - CHUNK_ROWS]
BATCH_SLOTS = 4            # slots per gather batch
GMAX = 96                  # padded graphs per core (real ~64)
SENT = 30_000.0            # one-hot sentinel (never matches iota 0..127)
_PAD_SPREAD = True         # pad gather slots: spread over 2048 rows vs row 0
OH_DVE_FRAC = 1.0          # fraction of one-hot build columns on DVE (rest Pool;
                           # plain TensorTensor is rejected on Pool by walrus)
ROTATE_QUEUES = True       # balance SWDGE queues across unequal chunk calls
SINGLE_PACKET = False      # dma_gather single_packet flag

F32 = mybir.dt.float32
BF16 = mybir.dt.bfloat16
I16 = mybir.dt.int16


# ---------------------------------------------------------------- host prep

def _prep(edge_index: np.ndarray, batch: np.ndarray):
    """All index-side preprocessing (sharding metadata). No float math on
    values here - only integer index manipulation derived from the graph
    structure, plus integer degree counts (rsqrt happens on device)."""
    src = np.asarray(edge_index[0], dtype=np.int64)
    dst = np.asarray(edge_index[1], dtype=np.int64)
    batch = np.asarray(batch, dtype=np.int64)

    deg = np.bincount(dst, minlength=N_NODES) + 1  # int degree (self-loop +1)

    gptr = np.searchsorted(batch, np.arange(N_GRAPHS + 1))
    targets = (np.arange(1, NCORES) * N_NODES) // NCORES
    gsplit = np.searchsorted(gptr, targets)
    g0 = np.concatenate([[0], gsplit, [N_GRAPHS]])
    n0s = gptr[g0]  # node range starts per core (len 9)

    order = np.argsort(dst, kind="stable")
    dst_s = dst[order]
    src_s = src[order]
    e0s = np.searchsorted(dst_s, n0s)

    cores = []
    for c in range(NCORES):
        n0, n1 = int(n0s[c]), int(n0s[c + 1])
        eso = src_s[e0s[c]:e0s[c + 1]]      # original (global) source ids
        ed = dst_s[e0s[c]:e0s[c + 1]]
        # self-loop edges are NOT in the gather stream: they are served by a
        # contiguous read of the core's own (rotation-fronted) table rows
        es = (eso - n0) % N_PAD             # rotated source rows
        slot = (ed - n0) >> 7
        chunk = np.searchsorted(np.array(CH_BOUNDS), es, side="right")
        o2 = np.lexsort((slot, chunk, slot >> 3))  # (batch, chunk, slot)
        cores.append({
            "n0": n0, "n1": n1, "g0": int(g0[c]), "g1": int(g0[c + 1]),
            "es": es[o2], "eso": eso[o2], "ed": ed[o2],
            "slot": slot[o2], "chunk": chunk[o2],
            "W": int(-(-(n1 - n0) // P)),
        })

    W_SLOTS = max(cr["W"] for cr in cores)
    W_SLOTS = -(-W_SLOTS // BATCH_SLOTS) * BATCH_SLOTS  # pad to batch multiple
    NB = W_SLOTS // BATCH_SLOTS

    # per (slot, chunk) 128-block counts, cross-core max
    counts = np.zeros((NCORES, W_SLOTS, NCHUNK), dtype=np.int64)
    for c, cr in enumerate(cores):
        key = cr["slot"] * NCHUNK + cr["chunk"]
        bc = np.bincount(key, minlength=W_SLOTS * NCHUNK)
        counts[c] = bc.reshape(W_SLOTS, NCHUNK)
    K = np.maximum(-(-counts.max(axis=0) // P), 1)  # [W_SLOTS, NCHUNK] blocks

    # block layout: ordered by (batch, chunk, slot-in-batch, k)
    # block_base[s, ch] = index of first block of that piece
    block_base = np.zeros((W_SLOTS, NCHUNK), dtype=np.int64)
    call_meta = []  # per gather call: (chunk, edge_base, n_edges)
    nb_blocks = 0
    for b in range(NB):
        sl = slice(b * BATCH_SLOTS, (b + 1) * BATCH_SLOTS)
        for ch in range(NCHUNK):
            kb = K[sl, ch]
            block_base[sl, ch] = nb_blocks + np.concatenate([[0], np.cumsum(kb)[:-1]])
            ncall = int(kb.sum()) * P
            call_meta.append((ch, nb_blocks * P, ncall))
            nb_blocks += int(kb.sum())
    NSUB = nb_blocks
    NE_LAY = NSUB * P  # total gather slots per layer

    # per-sub (block) metadata: slot position + start/stop flags, slot-major
    sub_of = []  # in processing order: (sub_idx, slot, is_first, is_last)
    for b in range(NB):
        for s in range(b * BATCH_SLOTS, (b + 1) * BATCH_SLOTS):
            subs = []
            for ch in range(NCHUNK):
                for k in range(int(K[s, ch])):
                    subs.append(int(block_base[s, ch]) + k)
            for i, sub in enumerate(subs):
                sub_of.append((sub, s, i == 0, i == len(subs) - 1))

    # per-core data arrays
    percore = []
    for c, cr in enumerate(cores):
        es, ed, slot, chunk = cr["es"], cr["ed"], cr["slot"], cr["chunk"]
        eso = cr["eso"]
        key = slot * NCHUNK + chunk
        # tiebreak by source row: descriptors within each piece ascend ->
        # locally monotone DMA address stream (even bank spread)
        grp_order = np.lexsort((es, key))
        # rank within (slot, chunk) group
        sort_key = key[grp_order]
        ranks = np.arange(len(es)) - np.searchsorted(sort_key, sort_key)
        # position of each edge
        pos = block_base[slot[grp_order], chunk[grp_order]] * P + ranks
        esg = es[grp_order]
        esog = eso[grp_order]
        edg = ed[grp_order]

        # pad slots gather garbage rows (masked by the one-hot sentinel);
        # spread them across each chunk's rows to avoid hammering one HBM
        # row with thousands of identical descriptors
        rng_pad = np.random.default_rng(12345)
        if _PAD_SPREAD:
            # spread pad descriptors across each call's whole chunk (the last
            # chunk is short); identical/clustered pad rows serialize the DMA
            # engines on HBM row conflicts
            idx_flat = np.empty(NE_LAY, dtype=np.int16)
            for ch_, ebase_, ncall_ in call_meta:
                idx_flat[ebase_:ebase_ + ncall_] = rng_pad.integers(
                    0, CHUNK_ROWS, size=ncall_).astype(np.int16)
        else:
            idx_flat = np.zeros(NE_LAY, dtype=np.int16)
        idx_flat[pos] = (esg - np.array(CH_STARTS)[chunk[grp_order]]).astype(np.int16)

        dst_rel = np.full((P, NSUB), SENT, dtype=np.float32)
        dst_rel[pos % P, pos >> 7] = (edg - cr["n0"] - slot[grp_order] * P).astype(np.float32)
        # slot-major column order so each slot's blocks are contiguous (the
        # batched one-hot build slices a contiguous range per slot)
        dst_rel = dst_rel[:, [so[0] for so in sub_of]]
        dst_rel = dst_rel.astype(mybir.dt.np(BF16))  # exact for 0..127 + sentinel

        deg_prod = np.ones((P, NSUB), dtype=np.float32)
        deg_prod[pos % P, pos >> 7] = (deg[esog] * deg[edg]).astype(np.float32)

        nown = cr["n1"] - cr["n0"]
        ar = np.arange(nown)
        deg_own = np.ones((P, W_SLOTS), dtype=np.float32)
        deg_own[ar % P, ar >> 7] = deg[cr["n0"]:cr["n1"]].astype(np.float32)

        g_rel = np.full((P, W_SLOTS), SENT, dtype=np.float32)
        g_rel[ar % P, ar >> 7] = (batch[cr["n0"]:cr["n1"]] - cr["g0"]).astype(np.float32)
        g_rel = g_rel.astype(mybir.dt.np(BF16))  # exact (values < 256 + sentinel)

        # pack idx into [128, NE_LAY//16] int16 col-major-16 replicated
        cols = NE_LAY // 16
        arr = np.zeros((16, cols), dtype=np.int16)
        j = np.arange(NE_LAY)
        arr[j % 16, j // 16] = idx_flat
        idx_packed = np.tile(arr, (8, 1))

        percore.append({
            **{k: cr[k] for k in ("n0", "n1", "g0", "g1", "W")},
            "idx_packed": idx_packed, "dst_rel": dst_rel,
            "deg_prod": deg_prod, "deg_own": deg_own, "g_rel": g_rel,
        })

    struct = {
        "W_SLOTS": W_SLOTS, "NB": NB, "NSUB": NSUB, "NE_LAY": NE_LAY,
        "K": K, "block_base": block_base, "call_meta": call_meta,
        "sub_of": sub_of,
    }
    return struct, percore, deg, gptr, n0s, g0


# ------------------------------------------------------------- bass program

def _build_launch(struct, layer: int, reps: int = 1, timing: bool = False, part: str = 'all'):
    """Build the SPMD Bass program for layer 1 or layer 2(+pool+mlp)."""
    W_SLOTS, NB = struct["W_SLOTS"], struct["NB"]
    NSUB, NE_LAY = struct["NSUB"], struct["NE_LAY"]
    K, block_base = struct["K"], struct["block_base"]
    call_meta, sub_of = struct["call_meta"], struct["sub_of"]

    nc = bacc.Bacc("TRN2", num_swdge_queues=4)
    table = nc.dram_tensor("table", (N_PAD, TBLW), BF16, kind="ExternalInput")
    idx_in = nc.dram_tensor("idx", (P, NE_LAY // 16), I16, kind="ExternalInput")
    dst_rel_in = nc.dram_tensor("dst_rel", (P, NSUB), BF16, kind="ExternalInput")
    deg_own_in = nc.dram_tensor("deg_own", (P, W_SLOTS), F32, kind="ExternalInput")
    iota_in = nc.dram_tensor("iota", (P, P), F32, kind="ExternalInput")
    ones_in = nc.dram_tensor("ones1", (1, P), F32, kind="ExternalInput")
    ident_in = nc.dram_tensor("ident", (P, P), F32, kind="ExternalInput")
    msg_w = 2 if layer == 1 else HID
    # conv weight with the bias folded in as a final row ([W; b])
    w_in = nc.dram_tensor("Wc", (msg_w + 1, HID), F32, kind="ExternalInput")
    if layer == 1:
        deg_prod_in = nc.dram_tensor("deg_prod", (P, NSUB), F32, kind="ExternalInput")
        if timing:
            out_t = nc.dram_tensor("h1s_scratch", (W_SLOTS * P, HID), BF16)
            dummy_t = nc.dram_tensor("tdummy0", (1, 4), F32, kind="ExternalOutput")
        else:
            out_t = nc.dram_tensor("h1s", (W_SLOTS * P, HID), BF16, kind="ExternalOutput")
    else:
        g_rel_in = nc.dram_tensor("g_rel", (P, W_SLOTS), BF16, kind="ExternalInput")
        deg_row_in = nc.dram_tensor("deg_row", (1, W_SLOTS * P), F32, kind="ExternalInput")
        wf1_in = nc.dram_tensor("Wf1", (HID, HID), F32, kind="ExternalInput")
        bf1_in = nc.dram_tensor("bf1", (1, HID), F32, kind="ExternalInput")
        wf2_in = nc.dram_tensor("Wf2", (HID, 4), F32, kind="ExternalInput")
        bf2_in = nc.dram_tensor("bf2", (1, 4), F32, kind="ExternalInput")
        if timing:
            out_t = nc.dram_tensor("out_scratch", (GMAX, 4), F32)
            dummy_t = nc.dram_tensor("tdummy0", (1, 4), F32, kind="ExternalOutput")
        else:
            out_t = nc.dram_tensor("out", (GMAX, 4), F32, kind="ExternalOutput")

    # organize subs per slot for slot-major processing
    slot_subs = [[] for _ in range(W_SLOTS)]
    for sub, s, first, last in sub_of:
        slot_subs[s].append(sub)
    # slot-major column starts into the (permuted) dst_rel array
    slot_col0 = np.zeros(W_SLOTS, dtype=np.int64)
    acc_cols = 0
    for s in range(W_SLOTS):
        slot_col0[s] = acc_cols
        acc_cols += len(slot_subs[s])

    # sub -> (call index, block-within-call) for gather tile slicing
    sub_call = np.zeros(NSUB, dtype=np.int64)
    sub_kloc = np.zeros(NSUB, dtype=np.int64)
    for ci, (ch, ebase, ncall) in enumerate(call_meta):
        b0 = ebase // P
        nb = ncall // P
        sub_call[b0:b0 + nb] = ci
        sub_kloc[b0:b0 + nb] = np.arange(nb)

    with tile.TileContext(nc) as tc:
        with tc.tile_pool(name="const", bufs=1) as cpool, \
             tc.tile_pool(name="meta", bufs=1) as mpool, \
             tc.tile_pool(name="gat", bufs=5) as gpool, \
             tc.tile_pool(name="own", bufs=3) as opool, \
             tc.tile_pool(name="gbf", bufs=8) as gbpool, \
             tc.tile_pool(name="work", bufs=3) as wpool, \
             tc.tile_pool(name="oh", bufs=3) as ohpool, \
             tc.tile_pool(name="pacc", bufs=3, space="PSUM") as pacc, \
             tc.tile_pool(name="ptp", bufs=2, space="PSUM") as ptp, \
             tc.tile_pool(name="ppool", bufs=1, space="PSUM") as ppool:

            # ---- load constants / metadata
            iota_t = cpool.tile([P, P], F32)
            nc.sync.dma_start(out=iota_t[:], in_=iota_in[:])
            iota_bf = cpool.tile([P, P], BF16)
            nc.vector.tensor_copy(out=iota_bf[:], in_=iota_t[:])
            ident_t = cpool.tile([P, P], F32)
            nc.sync.dma_start(out=ident_t[:], in_=ident_in[:])
            ident_bf = cpool.tile([P, P], BF16)
            nc.vector.tensor_copy(out=ident_bf[:], in_=ident_t[:])
            ones_t = cpool.tile([1, P], F32)
            nc.sync.dma_start(out=ones_t[:], in_=ones_in[:])
            w_t = cpool.tile([msg_w + 1, HID], F32)
            nc.sync.dma_start(out=w_t[:], in_=w_in[:])
            b_row_t = cpool.tile([1, HID], F32)
            nc.sync.dma_start(out=b_row_t[:], in_=w_in[msg_w:msg_w + 1, :])

            idx_t = mpool.tile([P, NE_LAY // 16], I16)
            nc.sync.dma_start(out=idx_t[:], in_=idx_in[:])
            dst_rel_t = mpool.tile([P, NSUB], BF16)
            nc.sync.dma_start(out=dst_rel_t[:], in_=dst_rel_in[:])
            deg_own_t = mpool.tile([P, W_SLOTS], F32)
            nc.sync.dma_start(out=deg_own_t[:], in_=deg_own_in[:])

            # dinv2 = 1/deg (self-loop weight), dinv = rsqrt(deg)
            dinv2_own_t = mpool.tile([P, W_SLOTS], F32)
            nc.vector.reciprocal(out=dinv2_own_t[:], in_=deg_own_t[:])
            dinv_own_t = deg_own_t
            nc.scalar.sqrt(out=dinv_own_t[:], in_=dinv2_own_t[:])

            if layer == 1:
                # per-edge norm = rsqrt(deg[src] * deg[dst]), in place
                deg_prod_t = mpool.tile([P, NSUB], F32)
                nc.sync.dma_start(out=deg_prod_t[:], in_=deg_prod_in[:])
                norm_t = deg_prod_t
                nc.vector.reciprocal(out=norm_t[:], in_=deg_prod_t[:])
                nc.scalar.sqrt(out=norm_t[:], in_=norm_t[:])
                # per-slot diag(dinv^2) for the self-loop matmul (bf16)
                diag2_all = cpool.tile([P, W_SLOTS, P], BF16)
                nc.vector.tensor_tensor(
                    out=diag2_all[:],
                    in0=ident_bf[:].rearrange("p (k c) -> p k c", k=1)
                        .broadcast_to((P, W_SLOTS, P)),
                    in1=dinv2_own_t[:].rearrange("p (k o) -> p k o", o=1)
                        .broadcast_to((P, W_SLOTS, P)),
                    op=mybir.AluOpType.mult)

            if layer == 2:
                # per-slot sqrt(deg) rows for the bias matmul (bias must not
                # be scaled by the deferred dinv: h = relu(dinv*(agg@W) + b)
                # is realized as relu(dinv*(agg@W + sqrtdeg*b)))
                deg_row_t = mpool.tile([1, W_SLOTS * P], F32)
                nc.sync.dma_start(out=deg_row_t[:], in_=deg_row_in[:])
                sqdeg_row_t = deg_row_t
                nc.scalar.sqrt(out=sqdeg_row_t[:], in_=deg_row_t[:])

            if layer == 2:
                g_rel_t = mpool.tile([P, W_SLOTS], BF16)
                nc.sync.dma_start(out=g_rel_t[:], in_=g_rel_in[:])
                wf1_t = cpool.tile([HID, HID], F32)
                nc.sync.dma_start(out=wf1_t[:], in_=wf1_in[:])
                wf2_t = cpool.tile([HID, 4], F32)
                nc.sync.dma_start(out=wf2_t[:], in_=wf2_in[:])
                bf1_t = cpool.tile([1, HID], F32)
                nc.sync.dma_start(out=bf1_t[:], in_=bf1_in[:])
                bf2_t = cpool.tile([1, 4], F32)
                nc.sync.dma_start(out=bf2_t[:], in_=bf2_in[:])
                # head bias broadcasts via ones-matmul
                bb2_ps = ptp.tile([P, HID], F32, space="PSUM", tag="hps")
                nc.tensor.matmul(out=bb2_ps[:], lhsT=ones_t[:], rhs=bf1_t[:],
                                 start=True, stop=True)
                bf1_bcast = cpool.tile([P, HID], F32)
                nc.vector.tensor_copy(out=bf1_bcast[:], in_=bb2_ps[:])
                bb3_ps = ptp.tile([P, 4], F32, space="PSUM", tag="hps")
                nc.tensor.matmul(out=bb3_ps[:], lhsT=ones_t[:], rhs=bf2_t[:],
                                 start=True, stop=True)
                bf2_bcast = cpool.tile([P, 4], F32)
                nc.vector.tensor_copy(out=bf2_bcast[:], in_=bb3_ps[:])
                pool_ps = ppool.tile([GMAX, HID + 1], F32, space="PSUM")

            # ---- main loop over batches (reps>1 repeats the whole
            # edge pass for timing-slope measurement; outputs stay valid
            # because each rep re-derives the same values).  part='gather'
            # instead repeats only the dma_gather calls (WAW-chained on the
            # same tile); part='compute' repeats only the scatter/compute.
            ncall_per_b = NCHUNK
            grep = reps if part == "gather" else 1
            crep = reps if part == "compute" else 1
            arep = reps if part == "all" else 1
            for rep, b in [(r, b) for r in range(arep) for b in range(NB)]:
                gtiles = []
                for ci in range(b * ncall_per_b, (b + 1) * ncall_per_b):
                    ch, ebase, ncall = call_meta[ci]
                    g_t = gpool.tile([P, ncall // P, TBLW], BF16, tag=f"g{ci % ncall_per_b}")
                    qn = (ci + ci // 4) % 4 if ROTATE_QUEUES else ci % 4
                    for _ in range(grep):
                        nc.gpsimd.dma_gather(
                            out_ap=g_t[:],
                            in_ap=table[CH_STARTS[ch]: CH_STARTS[ch] + CHUNK_ROWS, :],
                            idxs_ap=idx_t[:, ebase // 16:(ebase + ncall) // 16],
                            num_idxs=ncall, num_idxs_reg=ncall, elem_size=TBLW,
                            single_packet=SINGLE_PACKET, queue_num=qn)
                    if layer == 1:
                        # fuse per-edge norm into the (tiny 2-wide) messages
                        b0 = ebase // P
                        g_bf = gbpool.tile([P, ncall // P, msg_w], BF16, tag="gb")
                        nc.vector.tensor_tensor(
                            out=g_bf[:], in0=g_t[:, :, 0:msg_w],
                            in1=norm_t[:, b0:b0 + ncall // P]
                                .rearrange("p (k o) -> p k o", o=1)
                                .broadcast_to((P, ncall // P, msg_w)),
                            op=mybir.AluOpType.mult)
                        gtiles.append(g_bf)
                    else:
                        gtiles.append(g_t)

                if part == "gather":
                    continue
                if layer == 2:
                    goh_big = ohpool.tile([P, BATCH_SLOTS, GMAX], BF16, tag="goh")
                    nc.vector.tensor_tensor(
                        out=goh_big[:],
                        in0=iota_bf[:, 0:GMAX]
                            .rearrange("p (k c) -> p k c", k=1)
                            .broadcast_to((P, BATCH_SLOTS, GMAX)),
                        in1=g_rel_t[:, b * BATCH_SLOTS:(b + 1) * BATCH_SLOTS]
                            .rearrange("p (k o) -> p k o", o=1)
                            .broadcast_to((P, BATCH_SLOTS, GMAX)),
                        op=mybir.AluOpType.is_equal)
                for crep_i, s in [(r, s) for r in range(crep)
                                  for s in range(b * BATCH_SLOTS, (b + 1) * BATCH_SLOTS)]:
                    subs = slot_subs[s]
                    k = len(subs)
                    col0 = slot_col0[s]
                    # accumulate the TRANSPOSED aggregate: accT[f, d] so the
                    # conv matmul consumes it directly (no PE transpose)
                    acc = pacc.tile([msg_w, P], F32, space="PSUM", tag="acc")
                    # batched 0/1 one-hots for all this slot's blocks at once
                    oh_big = ohpool.tile([P, k, P], BF16, tag="oh")
                    nc.vector.tensor_tensor(
                        out=oh_big[:],
                        in0=iota_bf[:]
                            .rearrange("p (k c) -> p k c", k=1)
                            .broadcast_to((P, k, P)),
                        in1=dst_rel_t[:, col0:col0 + k]
                            .rearrange("p (k o) -> p k o", o=1)
                            .broadcast_to((P, k, P)),
                        op=mybir.AluOpType.is_equal)
                    for i, sub in enumerate(subs):
                        ci = int(sub_call[sub])
                        kloc = int(sub_kloc[sub])
                        g_t = gtiles[ci % ncall_per_b]
                        lhs = g_t[:, kloc, :] if layer == 1 else g_t[:, kloc, 0:HID]
                        nc.tensor.matmul(
                            out=acc[:], lhsT=lhs,
                            rhs=oh_big[:, i, :],
                            start=(i == 0), stop=False)
                    # self-loop term: this core's own rows are rotation-
                    # fronted at compile-time offsets -> contiguous DMA plus
                    # an identity (L2) / diag(dinv^2) (L1) matmul
                    own_t = opool.tile([P, TBLW], BF16, tag="own")
                    nc.sync.dma_start(out=own_t[:], in_=table[s * P:(s + 1) * P, :])
                    rhs_self = diag2_all[:, s, :] if layer == 1 else ident_bf[:]
                    nc.tensor.matmul(out=acc[:], lhsT=own_t[:, 0:msg_w],
                                     rhs=rhs_self, start=False, stop=True)

                    # ---- slot tail: evacuate accT, conv matmul + bias row
                    a2t = wpool.tile([msg_w, P], F32, tag="a2t")
                    nc.scalar.copy(out=a2t[:], in_=acc[:])
                    h_ps = ptp.tile([P, HID], F32, space="PSUM", tag="hps")
                    nc.tensor.matmul(out=h_ps[:], lhsT=a2t[:], rhs=w_t[0:msg_w, :],
                                     start=True, stop=False)
                    if layer == 1:
                        nc.tensor.matmul(out=h_ps[:], lhsT=ones_t[:],
                                         rhs=b_row_t[:],
                                         start=False, stop=True)
                        h1s = wpool.tile([P, HID], BF16, tag="h1s")
                        # emit dinv[node] * relu(conv1): dinv_src prefold for
                        # the layer-2 gather (bf16 table rows)
                        nc.scalar.activation(
                            out=h1s[:], in_=h_ps[:],
                            func=mybir.ActivationFunctionType.Relu,
                            scale=dinv_own_t[:, s:s + 1])
                        nc.sync.dma_start(out=out_t[s * P:(s + 1) * P, :], in_=h1s[:])
                    else:
                        # bias row scaled by sqrt(deg) so the dinv activation
                        # scale yields relu(dinv*(agg@W) + b)
                        nc.tensor.matmul(out=h_ps[:],
                                         lhsT=sqdeg_row_t[:, s * P:(s + 1) * P],
                                         rhs=b_row_t[:],
                                         start=False, stop=True)
                        h2 = wpool.tile([P, HID + 1], BF16, tag="h2")
                        nc.scalar.activation(
                            out=h2[:, 0:HID], in_=h_ps[:],
                            func=mybir.ActivationFunctionType.Relu,
                            scale=dinv_own_t[:, s:s + 1])
                        nc.vector.memset(h2[:, HID:HID + 1], 1.0)
                        nc.tensor.matmul(out=pool_ps[:],
                                         lhsT=goh_big[:, s - b * BATCH_SLOTS, :],
                                         rhs=h2[:],
                                         start=(s == 0), stop=(s == W_SLOTS - 1))

            if timing:
                d = wpool.tile([1, 4], F32, tag="dmy")
                nc.vector.memset(d[:], 0.0)
                nc.sync.dma_start(out=dummy_t[:], in_=d[:])

            # ---- pool + MLP head (layer 2)
            if layer == 2 and part not in ("gather",):
                pool_sb = wpool.tile([GMAX, HID + 1], F32, tag="pool")
                nc.vector.tensor_copy(out=pool_sb[:], in_=pool_ps[:])
                cnt = wpool.tile([GMAX, 1], F32, tag="cnt")
                nc.vector.tensor_scalar(
                    out=cnt[:], in0=pool_sb[:, HID:HID + 1], scalar1=1.0,
                    scalar2=None, op0=mybir.AluOpType.max)
                rcnt = wpool.tile([GMAX, 1], F32, tag="rcnt")
                nc.vector.reciprocal(out=rcnt[:], in_=cnt[:])
                means = wpool.tile([GMAX, HID], F32, tag="means")
                nc.scalar.mul(means[:], pool_sb[:, 0:HID], rcnt[:])
                mt_ps = ptp.tile([HID, GMAX], F32, space="PSUM", tag="tp")
                nc.tensor.transpose(out=mt_ps[:], in_=means[:],
                                    identity=ident_t[0:GMAX, 0:GMAX])
                mt = wpool.tile([HID, GMAX], F32, tag="mt")
                nc.vector.tensor_copy(out=mt[:], in_=mt_ps[:])
                f1_ps = ptp.tile([GMAX, HID], F32, space="PSUM", tag="hps")
                nc.tensor.matmul(out=f1_ps[:], lhsT=mt[:], rhs=wf1_t[:],
                                 start=True, stop=True)
                f1 = wpool.tile([GMAX, HID], F32, tag="f1")
                nc.vector.tensor_tensor(out=f1[:], in0=f1_ps[:],
                                        in1=bf1_bcast[0:GMAX, :],
                                        op=mybir.AluOpType.add)
                nc.scalar.activation(out=f1[:], in_=f1[:],
                                     func=mybir.ActivationFunctionType.Relu)
                f1t_ps = ptp.tile([HID, GMAX], F32, space="PSUM", tag="tp")
                nc.tensor.transpose(out=f1t_ps[:], in_=f1[:],
                                    identity=ident_t[0:GMAX, 0:GMAX])
                f1t = wpool.tile([HID, GMAX], F32, tag="f1t")
                nc.vector.tensor_copy(out=f1t[:], in_=f1t_ps[:])
                o_ps = ptp.tile([GMAX, 4], F32, space="PSUM", tag="hps")
                nc.tensor.matmul(out=o_ps[:], lhsT=f1t[:], rhs=wf2_t[:],
                                 start=True, stop=True)
                o_sb = wpool.tile([GMAX, 4], F32, tag="osb")
                nc.vector.tensor_tensor(out=o_sb[:], in0=o_ps[:],
                                        in1=bf2_bcast[0:GMAX, :],
                                        op=mybir.AluOpType.add)
                nc.sync.dma_start(out=out_t[:], in_=o_sb[:])

    nc.finalize()
    return nc


# ---------------------------------------------------------------- pjrt run

class _Runner:
    def __init__(self, nc, n_cores: int = NCORES):
        import jax
        from jax.sharding import Mesh, NamedSharding, PartitionSpec
        from jax.experimental.shard_map import shard_map
        from concourse.bass2jax import (
            _bass_exec_p, install_neuronx_cc_hook, partition_id_tensor)

        install_neuronx_cc_hook()
        self.jax = jax
        self.n_cores = n_cores
        in_names, out_names, out_avals = [], [], []
        pname = nc.partition_id_tensor.name if nc.partition_id_tensor else None
        for alloc in nc.m.functions[0].allocations:
            if not isinstance(alloc, mybir.MemoryLocationSet):
                continue
            name = alloc.memorylocations[0].name
            if alloc.kind == "ExternalInput":
                if name != pname:
                    in_names.append(name)
            elif alloc.kind == "ExternalOutput":
                out_names.append(name)
                out_avals.append(jax.core.ShapedArray(
                    tuple(alloc.tensor_shape), mybir.dt.np(alloc.dtype)))
        self.in_names, self.out_names, self.out_avals = in_names, out_names, out_avals
        n_params, n_outs = len(in_names), len(out_avals)
        all_in = in_names + out_names + ([pname] if pname else [])

        def _body(*args):
            operands = list(args)
            if pname:
                operands.append(partition_id_tensor())
            return tuple(_bass_exec_p.bind(
                *operands, out_avals=tuple(out_avals),
                in_names=tuple(all_in), out_names=tuple(out_names),
                lowering_input_output_aliases=(),
                sim_require_finite=True, sim_require_nnan=True, nc=nc))

        devices = jax.devices()[:n_cores]
        self.mesh = Mesh(np.asarray(devices), ("core",))
        self.sh = NamedSharding(self.mesh, PartitionSpec("core"))
        self.fn = jax.jit(
            shard_map(_body, mesh=self.mesh,
                      in_specs=(PartitionSpec("core"),) * (n_params + n_outs),
                      out_specs=(PartitionSpec("core"),) * n_outs,
                      check_rep=False),
            donate_argnums=tuple(range(n_params, n_params + n_outs)),
            keep_unused=True)
        self._zs = [(n_cores * a.shape[0], *a.shape[1:]) for a in out_avals]
        self._zd = [a.dtype for a in out_avals]
        self._dev_in = None

    def stage(self, in_maps):
        ci = [np.concatenate([np.ascontiguousarray(in_maps[c][n])
                              for c in range(self.n_cores)], axis=0)
              for n in self.in_names]
        self._dev_in = [self.jax.device_put(x, self.sh) for x in ci]
        for x in self._dev_in:
            x.block_until_ready()

    def run(self):
        zeros = [self.jax.device_put(np.zeros(s, d), self.sh)
                 for s, d in zip(self._zs, self._zd)]
        outs = self.fn(*self._dev_in, *zeros)
        for o in outs:
            o.block_until_ready()
        return outs

    def results(self, outs):
        res = []
        for c in range(self.n_cores):
            d = {}
            for i, n in enumerate(self.out_names):
                a = np.asarray(outs[i]).reshape(self.n_cores, *self.out_avals[i].shape)
                d[n] = a[c]
            res.append(d)
        return res


# ----------------------------------------------------------------- kernel()

_CACHE = {}

# timing info from the last kernel() call, for test.py
last_run_info = {}


def _consts():
    iota = np.tile(np.arange(P, dtype=np.float32), (P, 1))
    ident = np.eye(P, dtype=np.float32)
    ones1 = np.ones((1, P), dtype=np.float32)
    return iota, ident, ones1


def kernel(x, edge_index, batch, num_graphs=None, W1=None, b1=None, W2=None,
           b2=None, Wf1=None, bf1=None, Wf2=None, bf2=None):
    bfnp = mybir.dt.np(BF16)
    x = np.asarray(x, dtype=np.float32)
    W1 = np.asarray(W1, dtype=np.float32)
    b1 = np.asarray(b1, dtype=np.float32).reshape(1, HID)
    W2 = np.asarray(W2, dtype=np.float32)
    b2 = np.asarray(b2, dtype=np.float32).reshape(1, HID)
    Wf1 = np.asarray(Wf1, dtype=np.float32)
    bf1 = np.asarray(bf1, dtype=np.float32).reshape(1, HID)
    Wf2 = np.asarray(Wf2, dtype=np.float32)
    bf2 = np.asarray(bf2, dtype=np.float32).reshape(1, 4)
    W1c = np.concatenate([W1, b1], axis=0)  # [3, HID] folded bias
    W2c = np.concatenate([W2, b2], axis=0)  # [65, HID]

    ei = np.asarray(edge_index)
    bt = np.asarray(batch)
    key = hash((ei.tobytes(), bt.tobytes()))
    if key not in _CACHE:
        t0 = time.time()
        struct, percore, deg, gptr, n0s, g0 = _prep(ei, bt)
        nc1 = _build_launch(struct, 1)
        r1 = _Runner(nc1)
        nc2 = _build_launch(struct, 2)
        r2 = _Runner(nc2)
        _CACHE[key] = (struct, percore, r1, r2)
        last_run_info["build_s"] = time.time() - t0
    struct, percore, r1, r2 = _CACHE[key]

    iota, ident, ones1 = _consts()

    # launch 1: table = zero-padded x in bf16 (first 2 of 128 cols),
    # rotated per core so own rows are fronted (pure index movement)
    xpad = np.zeros((N_PAD, TBLW), dtype=bfnp)
    xpad[:N_NODES, 0:2] = x.astype(bfnp)
    rots = [np.concatenate([np.arange(pc["n0"], N_PAD), np.arange(0, pc["n0"])])
            for pc in percore]

    maps1 = []
    for c in range(NCORES):
        pc = percore[c]
        maps1.append({
            "table": xpad[rots[c]], "idx": pc["idx_packed"], "dst_rel": pc["dst_rel"],
            "deg_prod": pc["deg_prod"], "deg_own": pc["deg_own"],
            "iota": iota, "ident": ident, "ones1": ones1,
            "Wc": W1c,
        })
    t0 = time.time()
    r1.stage(maps1)
    last_run_info["stage1_s"] = time.time() - t0
    t0 = time.time()
    outs1 = r1.run()
    last_run_info["run1_s"] = time.time() - t0
    res1 = r1.results(outs1)

    # host reassembly of the hidden table (pure data movement)
    h1s_full = np.zeros((N_PAD, TBLW), dtype=bfnp)
    for c in range(NCORES):
        pc = percore[c]
        n0, n1 = pc["n0"], pc["n1"]
        h1s_full[n0:n1, 0:HID] = res1[c]["h1s"][0:n1 - n0]

    maps2 = []
    for c in range(NCORES):
        pc = percore[c]
        maps2.append({
            "table": h1s_full[rots[c]], "idx": pc["idx_packed"], "dst_rel": pc["dst_rel"],
            "deg_own": pc["deg_own"], "g_rel": pc["g_rel"],
            "deg_row": np.ascontiguousarray(pc["deg_own"].T).reshape(1, -1),
            "iota": iota, "ident": ident, "ones1": ones1,
            "Wc": W2c, "Wf1": Wf1, "bf1": bf1,
            "Wf2": Wf2, "bf2": bf2,
        })
    t0 = time.time()
    r2.stage(maps2)
    last_run_info["stage2_s"] = time.time() - t0
    t0 = time.time()
    outs2 = r2.run()
    last_run_info["run2_s"] = time.time() - t0
    res2 = r2.results(outs2)

    out = np.zeros((N_GRAPHS, 4), dtype=np.float32)
    for c in range(NCORES):
        pc = percore[c]
        out[pc["g0"]:pc["g1"]] = res2[c]["out"][0:pc["g1"] - pc["g0"]]

    last_run_info["runners"] = (r1, r2)
    last_run_info["maps"] = (maps1, maps2)
    return out


def _burst_time(rx, burst: int = 6, rounds: int = 10):
    """Median amortized wall time per dispatch over bursts.

    The fastest round is dropped (pipeline-warmth outlier), then the
    median of the rest is returned — the tunnel dispatch cost is noisy
    (+-1.5ms) and min-statistics systematically underestimate."""
    import time as _t
    rx.run()  # warm
    vals = []
    for _ in range(rounds):
        t0 = _t.perf_counter()
        outs = None
        for _ in range(burst):
            zeros = [rx.jax.device_put(np.zeros(sh, d), rx.sh)
                     for sh, d in zip(rx._zs, rx._zd)]
            outs = rx.fn(*rx._dev_in, *zeros)
        for o in outs:
            o.block_until_ready()
        vals.append((_t.perf_counter() - t0) / burst)
    vals = sorted(vals)[1:]
    return sorted(vals)[len(vals) // 2]


def measure_hw_ns(reps: int = 33, reps_lo: int = 9):
    """On-device exec time per launch via work-repetition slope.

    Host dispatch through the axon tunnel has a ~12ms per-dispatch cost
    (with ~+-1.5ms noise) that has nothing to do with device execution.
    To time the device work we build two timing variants of each launch
    (identical I/O; the whole edge pass + compute repeated `reps_lo` and
    `reps` times, re-deriving identical values) and use
      t_device(per pass) = (t(reps) - t(reps_lo)) / (reps - reps_lo)
    with median burst statistics.  Both rep counts are large enough that
    the bursts are device-bound, so the dispatch constant (and any
    RPC/device pipelining) cancels in the difference; a three-point
    linearity check of this estimator agreed within ~5%.  One pass of a
    launch is the full device work of that launch minus its one-time
    constant/metadata loads (a few MB of contiguous DMA), which we add
    back as an estimate from bytes at stream rate plus drain overhead."""
    struct, percore, r1, r2 = next(iter(_CACHE.values()))
    maps1, maps2 = last_run_info["maps"]

    detail = {}
    total = 0.0
    for layer, maps in ((1, maps1), (2, maps2)):
        ts = {}
        for r in (reps_lo, reps):
            key = ("timing", layer, r)
            if key not in _CACHE:
                nct = _build_launch(struct, layer, reps=r, timing=True)
                rx = _Runner(nct)
                rx.stage(maps)
                _CACHE[key] = rx
            ts[r] = _burst_time(_CACHE[key])
        slope = (ts[reps] - ts[reps_lo]) / (reps - reps_lo)
        # one-time device work not captured by the slope: constant /
        # index / metadata loads at launch start (contiguous DMA).
        fixed_bytes = sum(np.asarray(v).nbytes for k, v in maps[0].items()
                          if k != "table")
        fixed_s = fixed_bytes / 300e9 + 20e-6
        detail[f"launch{layer}"] = {
            "slope_us": round(slope * 1e6, 1),
            "tlo_us": round(ts[reps_lo] * 1e6, 1),
            "tR_us": round(ts[reps] * 1e6, 1),
            "fixed_us": round(fixed_s * 1e6, 1),
        }
        total += max(slope, 0.0) + fixed_s
    last_run_info["hw_detail"] = detail
    return total * 1e9
